# revision 42
# baseline (speedup 1.0000x reference)
"""Trainium2 Bass kernel for a dense pre-LN transformer block.

Problem: B=2, T=2048, C=1024, H=16 heads (d=64), FFN 4x, causal attention.

Parallelization over 8 NeuronCores (single SPMD program, one launch):
  - Attention: head-tensor-parallel. Core c computes heads {2c, 2c+1} for
    both batches: LN1 (replicated), Q/K/V projections, causal-block
    attention with unnormalized softmax (denominator via a ones-column in
    the value tile), reciprocal of the denominator computed sender-side.
  - FOUR AllToAlls (one per (batch, local-head)) redistribute attn^T from
    head-split to token-split; each overlaps the next attention unit or
    the early FFN work, so almost no collective time is exposed.
  - Post-A2A: core c owns tokens [256c, 256c+256) of BOTH batches:
    output projection + residual, LN2, FFN, residual.

Key implementation choices:
  - All [t,c] -> [c,t] transposes run on the DMA engines (xbar
    dma_start_transpose, bf16) instead of the PE: frees ~100us of PE time.
  - LayerNorm rsqrt = exp(-0.5*log(var+eps)) on the scalar engine so the
    whole kernel uses one activation table set (no table reload thrash).
  - g/beta of both LayerNorms are folded into the adjacent weight
    matrices host-side (bias rows enter via ones-row matmuls).
  - Softmax normalization: sender computes recip(den) (single-pass
    approx), the reciprocal rides the A2A as row 64; receiver applies it
    with one broadcast-DMA + multiply per batch (no expensive DVE
    reciprocal on broadcast data).
  - Causal masks multiply on GpSimd (otherwise idle), constants arrive in
    a handful of packed DMAs issued on the scalar queue so the x-tile DMAs
    lead the sync queue.
"""

import numpy as np
import ml_dtypes

B, T, C = 2, 2048, 1024
H, D = 16, 64
FF = 4 * C
EPS = 1e-5
NCORES = 8
TOK = 256   # tokens owned per core PER BATCH in the post-A2A phase
BT = B * T

_CACHE = {}
DEBUG = False


# --------------------------------------------------------------------------
# device program
# --------------------------------------------------------------------------
def _build_program():
    import concourse.bass as bass
    import concourse.mybir as mybir
    import concourse.tile as tile
    from concourse import bacc

    dt = mybir.dt
    f32 = dt.float32

    nc = bacc.Bacc("TRN2", target_bir_lowering=False, debug=False,
                   num_devices=NCORES)

    bf16 = dt.bfloat16
    x_full = nc.dram_tensor("x_full", [BT, C], f32, kind="ExternalInput")
    x_own = nc.dram_tensor("x_own", [2 * TOK, C], f32, kind="ExternalInput")
    wqkv = nc.dram_tensor("wqkv", [C, 384], bf16, kind="ExternalInput")
    cb = nc.dram_tensor("cb", [128, 35], f32, kind="ExternalInput")
    rows = nc.dram_tensor("rows", [1, 2176], bf16, kind="ExternalInput")
    masks = nc.dram_tensor("masks", [4, 128, 512], bf16, kind="ExternalInput")
    wproj = nc.dram_tensor("wproj", [C, C], bf16, kind="ExternalInput")
    w1 = nc.dram_tensor("w1", [C, FF], bf16, kind="ExternalInput")
    w2 = nc.dram_tensor("w2", [FF, C], bf16, kind="ExternalInput")
    out = nc.dram_tensor("out", [2 * TOK, C], f32, kind="ExternalOutput")
    if DEBUG:
        dh = nc.dram_tensor("dh", [128, 4 * C], bf16, kind="ExternalOutput")
        dhT = nc.dram_tensor("dhT", [128, 4 * 8 * 128], bf16,
                             kind="ExternalOutput")
        dq = nc.dram_tensor("dq", [128, T], bf16, kind="ExternalOutput")
        dk = nc.dram_tensor("dk", [128, T], bf16, kind="ExternalOutput")
        dva = nc.dram_tensor("dva", [128, 16 * 130], bf16, kind="ExternalOutput")
        da = nc.dram_tensor("da", [65, T], bf16, kind="ExternalOutput")
        din = nc.dram_tensor("din", [8, 65, TOK], bf16, kind="ExternalOutput")
        dout = nc.dram_tensor("dout", [8, 65, TOK], bf16, kind="ExternalOutput")
        dao = nc.dram_tensor("dao", [128, 8 * TOK], bf16, kind="ExternalOutput")
        dx2 = nc.dram_tensor("dx2", [128, 4 * C], f32, kind="ExternalOutput")
        dh2p = nc.dram_tensor("dh2p", [128, 8 * TOK], bf16, kind="ExternalOutput")
        df1 = nc.dram_tensor("df1", [128, 32 * 512], bf16, kind="ExternalOutput")

    with tile.TileContext(nc, num_cores=NCORES) as tc:
        _body(nc, tc, tile, mybir, bass, locals())
    nc.compile()
    return nc


def _rsqrt4(nc, mybir, pool, var_ap, n, name):
    """rsqrt(var + EPS) on DVE: quake bit-trick seed + 2 Newton steps.

    var_ap: [128, n] f32 (may be strided). Returns a [128, n] f32 tile.
    Avoids the scalar engine entirely so the activation table never leaves
    the exp set."""
    dt = mybir.dt
    f32, i32, u32 = dt.float32, dt.int32, dt.uint32
    OP = mybir.AluOpType
    vv = pool.tile([128, n], f32, tag="vv", bufs=2, name=f"vv_{name}")
    nc.vector.tensor_scalar_add(out=vv[:], in0=var_ap, scalar1=EPS)
    y = pool.tile([128, n], f32, tag="yy", bufs=2, name=f"yy_{name}")
    nc.vector.tensor_scalar(out=y[:].bitcast(u32), in0=vv[:].bitcast(u32),
                            scalar1=1, scalar2=None,
                            op0=OP.logical_shift_right)
    nc.vector.tensor_scalar(out=y[:].bitcast(i32), in0=y[:].bitcast(i32),
                            scalar1=0x5F3759DF, scalar2=-1,
                            op0=OP.subtract, op1=OP.mult)
    t = pool.tile([128, n], f32, tag="tt", bufs=2, name=f"tt_{name}")
    for _ in range(2):
        nc.vector.tensor_mul(t[:], y[:], y[:])
        nc.vector.scalar_tensor_tensor(out=t[:], in0=t[:], scalar=-0.5,
                                       in1=vv[:], op0=OP.mult, op1=OP.mult)
        nc.vector.scalar_tensor_tensor(out=y[:], in0=t[:], scalar=1.5,
                                       in1=y[:], op0=OP.add, op1=OP.mult)
    return y


def _body(nc, tc, tile, mybir, bass, io):
    dt = mybir.dt
    f32, f32r, bf16 = dt.float32, dt.float32r, dt.bfloat16
    AF = mybir.ActivationFunctionType
    OP = mybir.AluOpType

    x_full, x_own = io["x_full"], io["x_own"]
    wqkv, cb, rows, masks = io["wqkv"], io["cb"], io["rows"], io["masks"]
    wproj, w1, w2, out = io["wproj"], io["w1"], io["w2"], io["out"]

    # ---- persistent pools ----
    consts = tc.alloc_tile_pool(name="consts", bufs=1)
    persA = tc.alloc_tile_pool(name="persA", bufs=1)
    persD = tc.alloc_tile_pool(name="persD", bufs=1)
    dram = tc.alloc_tile_pool(name="dram", bufs=1, space="DRAM")

    # constants: few packed DMAs, issued on the scalar queue so that the
    # x-tile loads own the sync queue from t=0
    wqkv_sb = consts.tile([128, 8, 384], bf16, name="wqkv_sb")
    nc.scalar.dma_start(out=wqkv_sb[:],
                        in_=wqkv[:].rearrange("(cc p) d -> p cc d", p=128))
    cb_sb = consts.tile([128, 35], f32, name="cb_sb")
    nc.scalar.dma_start(out=cb_sb[:], in_=cb[:])
    rows_sb = consts.tile([1, 2176], bf16, name="rows_sb")
    nc.scalar.dma_start(out=rows_sb[:], in_=rows[:])
    mask_sb = consts.tile([128, 4, 512], bf16, name="mask_sb")
    nc.scalar.dma_start(out=mask_sb[:], in_=masks[:].rearrange("i p t -> p i t"))
    eps_sb = consts.tile([128, 1], f32, name="eps_sb")
    nc.vector.memset(eps_sb[:], EPS)
    onesf_sb = consts.tile([1, 128], f32, name="onesf_sb")
    nc.vector.memset(onesf_sb[:], 1.0)
    ones_b = rows_sb[0:1, 0:128]
    bproj_row = rows_sb[0:1, 128:1152]
    b2_row = rows_sb[0:1, 1152:2176]

    # phase-2 weight prefetched from t=0
    wp_sb = persD.tile([128, 8, C], bf16, name="wp_sb")
    nc.scalar.dma_start(out=wp_sb[:],
                        in_=wproj[:].rearrange("(dc p) e -> p dc e", p=128))

    # attention-persistent tensors
    qT = [persA.tile([128, T], bf16, name=f"qTb{b}") for b in range(2)]
    kT = [persA.tile([128, T], bf16, name=f"kTb{b}") for b in range(2)]
    vaug = [persA.tile([128, 16, 130], bf16, name=f"vaugb{b}") for b in range(2)]
    for b in range(2):
        nc.vector.memset(vaug[b][:, :, 64:65], 1.0)
        nc.vector.memset(vaug[b][:, :, 129:130], 1.0)

    a2a_in = [dram.tile([8, 65, TOK], bf16, name=f"a2a_in{k}") for k in range(4)]
    a2a_out = [dram.tile([8, 65, TOK], bf16, name=f"a2a_out{k}") for k in range(4)]

    w1r = w1[:].rearrange("(cc p) m -> p cc m", p=128)
    psQ = tc.alloc_tile_pool(name="psM", bufs=1, space="PSUM")

    # ======================================================================
    # Phase A: LN1 + Q/K/V per batch  (hT via DMA-xbar transpose)
    # ======================================================================
    with tc.tile_pool(name="lnq", bufs=1) as lnp:
        aT = [lnp.tile([65, T], bf16, tag="aT", bufs=2, name=f"aT{k}")
              for k in range(4)]
        for b in range(2):
            vT = lnp.tile([128, T], bf16, tag="vT", bufs=1, name=f"vT_{b}")
            with nc.named_scope(f"qkv_b{b}"):
                for tch in range(4):
                    h = lnp.tile([128, 4, C], bf16, tag="h", bufs=2,
                                 name=f"h_{b}_{tch}")
                    mvt = lnp.tile([128, 2, 4], f32, tag="mv", bufs=2,
                                   name=f"mvt_{b}_{tch}")
                    xts = []
                    for ht in range(2):
                        row0 = b * T + tch * 512 + ht * 256
                        xt = lnp.tile([128, 2, C], f32, tag="xt", bufs=2,
                                      name=f"xt_{b}_{tch}_{ht}")
                        xts.append(xt)
                        nc.sync.dma_start(
                            out=xt[:],
                            in_=x_full[row0:row0 + 256, :].rearrange(
                                "(s p) c -> p s c", p=128))
                        for s2 in range(2):
                            s = 2 * ht + s2
                            st = lnp.tile([128, 2, 6], f32, tag="st", bufs=2,
                                          name=f"st_{b}_{tch}_{s}")
                            nc.vector.bn_stats(out=st[:, 0, :],
                                               in_=xt[:, s2, 0:512])
                            nc.vector.bn_stats(out=st[:, 1, :],
                                               in_=xt[:, s2, 512:1024])
                            nc.vector.bn_aggr(out=mvt[:, :, s], in_=st[:])
                    # rsqrt(var+eps) for all 4 subs: quake seed + 2 Newton
                    # steps, entirely on DVE (keeps ACT on one table set)
                    rsq = _rsqrt4(nc, mybir, lnp, mvt[:, 1, :], 4,
                                  f"{b}_{tch}")
                    for ht in range(2):
                        for s2 in range(2):
                            s = 2 * ht + s2
                            nc.vector.tensor_scalar(out=h[:, s, :],
                                                    in0=xts[ht][:, s2, :],
                                                    scalar1=mvt[:, 0, s:s + 1],
                                                    scalar2=rsq[:, s:s + 1],
                                                    op0=OP.subtract, op1=OP.mult)
                    # hT chunks via xbar DMA transpose (contiguous dst per s)
                    hTb = lnp.tile([128, 4, 8, 128], bf16, tag="hTb", bufs=1,
                                   name=f"hTb_{b}_{tch}")
                    for s in range(4):
                        nc.sync.dma_start_transpose(out=hTb[:, s], in_=h[:, s, :])
                    # repack to token-contiguous [cc, 512] via SBUF DMAs
                    hTq = lnp.tile([128, 8, 4, 128], bf16, tag="hTq", bufs=2,
                                   name=f"hTq_{b}_{tch}")
                    for s in range(4):
                        nc.scalar.dma_start(out=hTq[:, :, s, :],
                                            in_=hTb[:, s])
                    if DEBUG and b == 0 and tch == 0:
                        nc.sync.dma_start(
                            out=io["dh"][:],
                            in_=h[:].rearrange("p a b -> p (a b)"))
                        nc.sync.dma_start(
                            out=io["dhT"][:],
                            in_=hTb[:].rearrange("p a b c -> p (a b c)"))
                    col = tch * 512
                    # q^T, k^T, v^T  (full N=512 moving operand)
                    for wi, dst in ((0, qT[b]), (1, kT[b]), (2, vT)):
                        pqk = psQ.tile([128, 512], f32, tag="pqv", bufs=2,
                                       name=f"pq_{b}_{tch}_{wi}")
                        for cc in range(8):
                            nc.tensor.matmul(
                                pqk[:],
                                wqkv_sb[:, cc, wi * 128:(wi + 1) * 128],
                                hTq[:, cc],
                                start=(cc == 0), stop=(cc == 7))
                        nc.vector.tensor_scalar_add(out=dst[:, col:col + 512],
                                                    in0=pqk[:],
                                                    scalar1=cb_sb[:, wi:wi + 1])
                # v back to [t, d] layout: one full-tile xbar transpose,
                # then split into the per-head [.. 65 ..] stationary layout
                vfull = lnp.tile([128, 16, 128], bf16, tag="vfull", bufs=1,
                                 name=f"vfull_{b}")
                nc.sync.dma_start_transpose(out=vfull[:], in_=vT[:])
                nc.vector.tensor_copy(out=vaug[b][:, :, 0:64],
                                      in_=vfull[:, :, 0:64])
                nc.vector.tensor_copy(out=vaug[b][:, :, 65:129],
                                      in_=vfull[:, :, 64:128])

        # ==================================================================
        # Phase B: causal attention per (batch, local head) + its A2A
        # ==================================================================
        for b in range(2):
            for hh in range(2):
                k4 = 2 * b + hh
                hp = 64 * hh
                vs = 65 * hh
                with nc.named_scope(f"attn_b{b}h{hh}"):
                    for half in range(2):
                        qc0 = half * 1024
                        pat = [psQ.tile([128, 512], f32, tag=f"pat{i}", bufs=1,
                                        name=f"pat_{k4}_{half}_{i}")
                               for i in range(2)]
                        nsb = 8 * half + 8
                        for sb in range(nsb):
                            act0 = 0 if sb < 8 * half + 4 else 1
                            dtc = sb // 4 - 2 * half
                            ps = psQ.tile([128, 1024], f32, tag="ps", bufs=2,
                                          name=f"ps_{k4}_{half}_{sb}")
                            for i in range(act0, 2):
                                nc.tensor.matmul(
                                    ps[:, i * 512:(i + 1) * 512],
                                    kT[b][hp:hp + 64, sb * 128:sb * 128 + 128],
                                    qT[b][hp:hp + 64,
                                          qc0 + i * 512:qc0 + (i + 1) * 512],
                                    start=True, stop=True)
                            pt = lnp.tile([128, 1024], bf16, tag="pt", bufs=3,
                                          name=f"pt_{k4}_{half}_{sb}")
                            nc.scalar.activation(out=pt[:, act0 * 512:1024],
                                                 in_=ps[:, act0 * 512:1024],
                                                 func=AF.Exp, scale=0.125)
                            if dtc >= act0:
                                nc.vector.tensor_mul(
                                    pt[:, dtc * 512:(dtc + 1) * 512],
                                    pt[:, dtc * 512:(dtc + 1) * 512],
                                    mask_sb[:, sb % 4, :])
                            for i in range(act0, 2):
                                last = 8 * half + 3 if i == 0 else nsb - 1
                                nc.tensor.matmul(
                                    pat[i][0:65, :], vaug[b][:, sb, vs:vs + 65],
                                    pt[:, i * 512:(i + 1) * 512],
                                    start=(sb == 0), stop=(sb == last))
                        for i in range(2):
                            qcol = (2 * half + i) * 512
                            nc.vector.tensor_copy(
                                out=aT[k4][64:65, qcol:qcol + 512],
                                in_=pat[i][64:65, :])
                            nc.vector.tensor_copy(
                                out=aT[k4][0:64, qcol:qcol + 512],
                                in_=pat[i][0:64, :])
                if DEBUG and k4 == 0:
                    nc.sync.dma_start(out=io["dq"][:], in_=qT[0][:])
                    nc.sync.dma_start(out=io["dk"][:], in_=kT[0][:])
                    nc.sync.dma_start(
                        out=io["dva"][:],
                        in_=vaug[0][:].rearrange("p a b -> p (a b)"))
                    nc.sync.dma_start(out=io["da"][:], in_=aT[0][:])
                # ship: shard j = tokens [256j, 256j+256) of this (b, head)
                nc.sync.dma_start(
                    out=a2a_in[k4][:].rearrange("j d t -> d j t"),
                    in_=aT[k4][:].rearrange("d (j t) -> d j t", j=8))
                nc.gpsimd.collective_compute(
                    "AllToAll", mybir.AluOpType.bypass,
                    replica_groups=[list(range(NCORES))],
                    ins=[a2a_in[k4][:].opt()], outs=[a2a_out[k4][:].opt()])
                if DEBUG and k4 == 0:
                    nc.sync.dma_start(out=io["din"][:], in_=a2a_in[0][:])
                    nc.sync.dma_start(out=io["dout"][:], in_=a2a_out[0][:])

    # ======================================================================
    # Phase C: per batch: unpack A2A + normalize + proj + LN2 + FFN1
    # ======================================================================
    x2 = persD.tile([128, 4, C], f32, name="x2")
    ff1T = persD.tile([128, 32, 512], bf16, name="ff1T")
    h2T = [persD.tile([128, 2, 8, 128], bf16, name=f"h2T_{b}") for b in range(2)]

    for b in range(2):
        tcol = TOK * b
        with tc.tile_pool(name=f"prj{b}", bufs=1) as prp:
            xo = prp.tile([128, 2, C], f32, tag="xo", name=f"xo_{b}")
            nc.gpsimd.dma_start(
                out=xo[:],
                in_=x_own[TOK * b:TOK * (b + 1), :].rearrange(
                    "(q p) c -> p q c", p=128))
            aT_own = prp.tile([128, 8, TOK], bf16, tag="aTo", name=f"aTo_{b}")
            h2p = prp.tile([128, 8, TOK], bf16, tag="h2p", name=f"h2p_{b}")
            rb = prp.tile([128, 8, TOK], bf16, tag="rb", name=f"rb_{b}")
            for hh in range(2):
                k4 = 2 * b + hh
                nc.sync.dma_start(
                    out=aT_own[64 * hh:64 * hh + 64, :, :],
                    in_=a2a_out[k4][:, 0:64, :].rearrange("r d t -> d r t"))
                for r in range(8):
                    nc.sync.dma_start(
                        out=rb[64 * hh:64 * hh + 64, r, :],
                        in_=a2a_out[k4][r, 64:65, :].to_broadcast([64, TOK]))
            for r in range(8):
                rbf = prp.tile([128, TOK], f32, tag="rbf", bufs=2,
                               name=f"rbf_{b}_{r}")
                nc.vector.tensor_copy(out=rbf[:], in_=rb[:, r, :])
                rbf2 = prp.tile([128, TOK], f32, tag="rbf2", bufs=2,
                                name=f"rbf2_{b}_{r}")
                nc.vector.reciprocal_approx_fast(out=rbf2[:], in_=rbf[:])
                nc.vector.tensor_mul(aT_own[:, r, :], aT_own[:, r, :],
                                     rbf2[:])
            if DEBUG and b == 0:
                nc.sync.dma_start(
                    out=io["dao"][:],
                    in_=aT_own[:].rearrange("p a b -> p (a b)"))
            h2 = prp.tile([128, 2, C], bf16, tag="h2", name=f"h2_{b}")
            with nc.named_scope(f"proj_ln2_b{b}"):
                mv2t = prp.tile([128, 2, 2], f32, tag="mv2", bufs=2,
                                name=f"mv2_{b}")
                for tqi in range(2):
                    tq = 2 * b + tqi
                    for eh in range(2):
                        pp = psQ.tile([128, 512], f32, tag="pqv", bufs=2,
                                      name=f"pp_{tq}_{eh}")
                        for dc in range(8):
                            nc.tensor.matmul(
                                pp[:], aT_own[:, dc, tqi * 128:(tqi + 1) * 128],
                                wp_sb[:, dc, eh * 512:eh * 512 + 512],
                                start=(dc == 0), stop=False)
                        nc.tensor.matmul(pp[:], ones_b,
                                         bproj_row[0:1, eh * 512:eh * 512 + 512],
                                         start=False, stop=True)
                        nc.vector.tensor_add(x2[:, tq, eh * 512:eh * 512 + 512],
                                             pp[:], xo[:, tqi, eh * 512:eh * 512 + 512])
                    st2 = prp.tile([128, 2, 6], f32, tag="st2", bufs=2,
                                   name=f"st2_{tq}")
                    nc.vector.bn_stats(out=st2[:, 0, :], in_=x2[:, tq, 0:512])
                    nc.vector.bn_stats(out=st2[:, 1, :], in_=x2[:, tq, 512:1024])
                    nc.vector.bn_aggr(out=mv2t[:, :, tqi], in_=st2[:])
                rsq2 = _rsqrt4(nc, mybir, prp, mv2t[:, 1, :], 2, f"ln2_{b}")
                for tqi in range(2):
                    tq = 2 * b + tqi
                    nc.vector.tensor_scalar(out=h2[:, tqi, :], in0=x2[:, tq, :],
                                            scalar1=mv2t[:, 0, tqi:tqi + 1],
                                            scalar2=rsq2[:, tqi:tqi + 1],
                                            op0=OP.subtract, op1=OP.mult)
                    nc.sync.dma_start_transpose(out=h2T[b][:, tqi],
                                                in_=h2[:, tqi, :])
                # repack [tq, cc, 128] -> [cc, 256] (contiguous moving
                # operand for FFN1)
                for tqi in range(2):
                    nc.vector.tensor_copy(
                        out=h2p[:, :, tqi * 128:(tqi + 1) * 128],
                        in_=h2T[b][:, tqi])
            with nc.named_scope(f"ffn1_b{b}"):
                for w in range(16):
                    w1w = prp.tile([128, 8, 256], bf16, tag="w1w", bufs=3,
                                   name=f"w1w_{b}_{w}")
                    nc.gpsimd.dma_start(out=w1w[:],
                                        in_=w1r[:, :, w * 256:(w + 1) * 256])
                    for m2 in range(2):
                        m = w * 2 + m2
                        pf = psQ.tile([128, 512], f32, tag="pqv", bufs=2,
                                      name=f"pf_{b}_{m}")
                        for cc in range(8):
                            nc.tensor.matmul(
                                pf[:, 0:256], w1w[:, cc, m2 * 128:(m2 + 1) * 128],
                                h2p[:, cc, :],
                                start=(cc == 0), stop=(cc == 7))
                        nc.scalar.activation(out=ff1T[:, m, tcol:tcol + TOK],
                                             in_=pf[:, 0:256], func=AF.Relu,
                                             bias=cb_sb[:, 3 + m:4 + m])
            if DEBUG and b == 0:
                nc.sync.dma_start(
                    out=io["dh2p"][:],
                    in_=h2p[:].rearrange("p a b -> p (a b)"))
    if DEBUG:
        nc.sync.dma_start(out=io["dx2"][:],
                          in_=x2[:].rearrange("p a b -> p (a b)"))
        nc.sync.dma_start(out=io["df1"][:],
                          in_=ff1T[:].rearrange("p a b -> p (a b)"))

    psQ.release()

    # ======================================================================
    # Phase D: FFN2 over all 4 token-quarters, single w2 stream
    # ======================================================================
    with tc.tile_pool(name="ffn2p", bufs=1) as f2p, \
         tc.tile_pool(name="ps2", bufs=1, space="PSUM") as ps2p, \
         nc.named_scope("ffn2"):
        pso = [ps2p.tile([128, C], f32, tag="pso", bufs=4, name=f"pso_{tq}")
               for tq in range(4)]
        for mc in range(32):
            w2t = f2p.tile([128, C], bf16, tag="w2t", bufs=6, name=f"w2t_{mc}")
            nc.gpsimd.dma_start(out=w2t[:], in_=w2[mc * 128:(mc + 1) * 128, :])
            for tq in range(4):
                for eh in range(2):
                    nc.tensor.matmul(pso[tq][:, eh * 512:(eh + 1) * 512],
                                     ff1T[:, mc, tq * 128:(tq + 1) * 128],
                                     w2t[:, eh * 512:(eh + 1) * 512],
                                     start=(mc == 0), stop=False)
        for tq in range(4):
            for eh in range(2):
                nc.tensor.matmul(pso[tq][:, eh * 512:(eh + 1) * 512],
                                 ones_b, b2_row[0:1, eh * 512:(eh + 1) * 512],
                                 start=False, stop=True)
            ot = f2p.tile([128, C], f32, tag="ot", bufs=2, name=f"ot_{tq}")
            nc.vector.tensor_add(ot[:], pso[tq][:], x2[:, tq, :])
            nc.gpsimd.dma_start(out=out[tq * 128:(tq + 1) * 128, :], in_=ot[:])
    persD.release()
    persA.release()
    consts.release()
    dram.release()


# --------------------------------------------------------------------------
# host driver
# --------------------------------------------------------------------------
def _make_in_maps(inputs):
    bf = ml_dtypes.bfloat16
    x = np.ascontiguousarray(np.asarray(inputs["x"], np.float32))
    wq = np.asarray(inputs["wq"], np.float32)
    wk = np.asarray(inputs["wk"], np.float32)
    wv = np.asarray(inputs["wv"], np.float32)
    w_proj = np.asarray(inputs["w_proj"], np.float32)
    b_proj = np.asarray(inputs["b_proj"], np.float32)
    w1 = np.asarray(inputs["w1"], np.float32)
    b1 = np.asarray(inputs["b1"], np.float32)
    w2 = np.asarray(inputs["w2"], np.float32)
    b2 = np.asarray(inputs["b2"], np.float32)
    g1 = np.asarray(inputs["g1"], np.float32)
    be1 = np.asarray(inputs["be1"], np.float32)
    g2 = np.asarray(inputs["g2"], np.float32)
    be2 = np.asarray(inputs["be2"], np.float32)

    xf = x.reshape(BT, C)
    i_mask = np.zeros((4, 128, 512), np.float32)
    s_idx = np.arange(128)[:, None]
    t_idx = np.arange(512)[None, :]
    for i in range(4):
        i_mask[i] = (s_idx + 128 * i <= t_idx).astype(np.float32)

    w1f = g2[:, None] * w1                       # fold LN2 gamma
    b1f = b1 + be2 @ w1                          # fold LN2 beta
    rows = np.concatenate([np.ones(128, np.float32), b_proj, b2])[None, :]

    common = dict(
        x_full=xf,
        masks=i_mask.astype(bf),
        rows=rows.astype(bf),
        wproj=np.ascontiguousarray(w_proj).astype(bf),
        w1=np.ascontiguousarray(w1f).astype(bf),
        w2=np.ascontiguousarray(w2).astype(bf),
    )
    in_maps = []
    for c in range(NCORES):
        wqp = np.concatenate([wq[2 * c], wq[2 * c + 1]], axis=1)  # [C,128]
        wkp = np.concatenate([wk[2 * c], wk[2 * c + 1]], axis=1)
        wvp = np.concatenate([wv[2 * c], wv[2 * c + 1]], axis=1)
        wqkv = np.concatenate([g1[:, None] * wqp, g1[:, None] * wkp,
                               g1[:, None] * wvp], axis=1)        # [C,384]
        cbm = np.zeros((128, 35), np.float32)
        cbm[:, 0] = be1 @ wqp
        cbm[:, 1] = be1 @ wkp
        cbm[:, 2] = be1 @ wvp
        cbm[:, 3:35] = np.ascontiguousarray(b1f.reshape(32, 128).T)
        m = dict(common)
        m["wqkv"] = np.ascontiguousarray(wqkv).astype(bf)
        m["cb"] = np.ascontiguousarray(cbm)
        m["x_own"] = np.ascontiguousarray(
            np.concatenate([xf[TOK * c:TOK * (c + 1)],
                            xf[T + TOK * c:T + TOK * (c + 1)]], axis=0))
        in_maps.append(m)
    return in_maps


LAST_RESULTS = None


def kernel(trace=False, **inputs):
    global LAST_RESULTS
    from concourse import bass_utils

    if "nc" not in _CACHE:
        _CACHE["nc"] = _build_program()
    nc = _CACHE["nc"]
    in_maps = _make_in_maps(inputs)
    res = bass_utils.run_bass_kernel_spmd(
        nc, in_maps, core_ids=list(range(NCORES)), trace=trace)
    LAST_RESULTS = res
    out = np.zeros((B, T, C), np.float32)
    for c in range(NCORES):
        r = res.results[c]["out"]
        out[0, TOK * c:TOK * (c + 1), :] = r[0:TOK]
        out[1, TOK * c:TOK * (c + 1), :] = r[TOK:2 * TOK]
    return out


# revision 44
# speedup vs baseline: 1.0948x; 1.0948x over previous
"""Trainium2 Bass kernel for a dense pre-LN transformer block.

Problem: B=2, T=2048, C=1024, H=16 heads (d=64), FFN 4x, causal attention.

Parallelization over 8 NeuronCores (single SPMD program, one launch):
  - Attention: head-tensor-parallel. Core c computes heads {2c, 2c+1} for
    both batches: LN1 (replicated), Q/K/V projections, causal-block
    attention with unnormalized softmax (denominator via a ones-column in
    the value tile), reciprocal of the denominator computed sender-side.
  - FOUR AllToAlls (one per (batch, local-head)) redistribute attn^T from
    head-split to token-split; each overlaps the next attention unit or
    the early FFN work, so almost no collective time is exposed.
  - Post-A2A: core c owns tokens [256c, 256c+256) of BOTH batches:
    output projection + residual, LN2, FFN, residual.

Key implementation choices:
  - All [t,c] -> [c,t] transposes run on the DMA engines (xbar
    dma_start_transpose, bf16) instead of the PE: frees ~100us of PE time.
  - LayerNorm rsqrt = exp(-0.5*log(var+eps)) on the scalar engine so the
    whole kernel uses one activation table set (no table reload thrash).
  - g/beta of both LayerNorms are folded into the adjacent weight
    matrices host-side (bias rows enter via ones-row matmuls).
  - Softmax normalization: sender computes recip(den) (single-pass
    approx), the reciprocal rides the A2A as row 64; receiver applies it
    with one broadcast-DMA + multiply per batch (no expensive DVE
    reciprocal on broadcast data).
  - Causal masks multiply on GpSimd (otherwise idle), constants arrive in
    a handful of packed DMAs issued on the scalar queue so the x-tile DMAs
    lead the sync queue.
"""

import numpy as np
import ml_dtypes

B, T, C = 2, 2048, 1024
H, D = 16, 64
FF = 4 * C
EPS = 1e-5
NCORES = 8
TOK = 256   # tokens owned per core PER BATCH in the post-A2A phase
BT = B * T

_CACHE = {}
DEBUG = False


# --------------------------------------------------------------------------
# device program
# --------------------------------------------------------------------------
def _build_program():
    import concourse.bass as bass
    import concourse.mybir as mybir
    import concourse.tile as tile
    from concourse import bacc

    dt = mybir.dt
    f32 = dt.float32

    nc = bacc.Bacc("TRN2", target_bir_lowering=False, debug=False,
                   num_devices=NCORES)

    bf16 = dt.bfloat16
    x_full = nc.dram_tensor("x_full", [BT, C], f32, kind="ExternalInput")
    x_bf = nc.dram_tensor("x_bf", [BT, C], bf16, kind="ExternalInput")
    x_own = nc.dram_tensor("x_own", [2 * TOK, C], f32, kind="ExternalInput")
    wqkv = nc.dram_tensor("wqkv", [C, 384], bf16, kind="ExternalInput")
    cb = nc.dram_tensor("cb", [128, 32], f32, kind="ExternalInput")
    rows = nc.dram_tensor("rows", [1, 2944], bf16, kind="ExternalInput")
    masks = nc.dram_tensor("masks", [4, 128, 512], bf16, kind="ExternalInput")
    wproj = nc.dram_tensor("wproj", [C, C], bf16, kind="ExternalInput")
    w1 = nc.dram_tensor("w1", [C, FF], bf16, kind="ExternalInput")
    w2 = nc.dram_tensor("w2", [FF, C], bf16, kind="ExternalInput")
    out = nc.dram_tensor("out", [2 * TOK, C], f32, kind="ExternalOutput")
    if DEBUG:
        dh = nc.dram_tensor("dh", [128, 4 * C], bf16, kind="ExternalOutput")
        dhT = nc.dram_tensor("dhT", [128, 4 * 8 * 128], bf16,
                             kind="ExternalOutput")
        dq = nc.dram_tensor("dq", [128, T], bf16, kind="ExternalOutput")
        dk = nc.dram_tensor("dk", [128, T], bf16, kind="ExternalOutput")
        dva = nc.dram_tensor("dva", [128, 16 * 130], bf16, kind="ExternalOutput")
        da = nc.dram_tensor("da", [65, T], bf16, kind="ExternalOutput")
        din = nc.dram_tensor("din", [8, 65, TOK], bf16, kind="ExternalOutput")
        dout = nc.dram_tensor("dout", [8, 65, TOK], bf16, kind="ExternalOutput")
        dao = nc.dram_tensor("dao", [128, 8 * TOK], bf16, kind="ExternalOutput")
        dx2 = nc.dram_tensor("dx2", [128, 4 * C], f32, kind="ExternalOutput")
        dh2p = nc.dram_tensor("dh2p", [128, 8 * TOK], bf16, kind="ExternalOutput")
        df1 = nc.dram_tensor("df1", [128, 32 * 512], bf16, kind="ExternalOutput")

    with tile.TileContext(nc, num_cores=NCORES) as tc:
        _body(nc, tc, tile, mybir, bass, locals())
    nc.compile()
    return nc


def _rsqrt4(nc, mybir, pool, var_ap, n, name):
    """rsqrt(var + EPS) on DVE: quake bit-trick seed + 2 Newton steps.

    var_ap: [128, n] f32 (may be strided). Returns a [128, n] f32 tile.
    Avoids the scalar engine entirely so the activation table never leaves
    the exp set."""
    dt = mybir.dt
    f32, i32, u32 = dt.float32, dt.int32, dt.uint32
    OP = mybir.AluOpType
    vv = pool.tile([128, n], f32, tag="vv", bufs=2, name=f"vv_{name}")
    nc.vector.tensor_scalar_add(out=vv[:], in0=var_ap, scalar1=EPS)
    y = pool.tile([128, n], f32, tag="yy", bufs=2, name=f"yy_{name}")
    nc.vector.tensor_scalar(out=y[:].bitcast(u32), in0=vv[:].bitcast(u32),
                            scalar1=1, scalar2=None,
                            op0=OP.logical_shift_right)
    nc.vector.tensor_scalar(out=y[:].bitcast(i32), in0=y[:].bitcast(i32),
                            scalar1=0x5F3759DF, scalar2=-1,
                            op0=OP.subtract, op1=OP.mult)
    t = pool.tile([128, n], f32, tag="tt", bufs=2, name=f"tt_{name}")
    for _ in range(2):
        nc.vector.tensor_mul(t[:], y[:], y[:])
        nc.vector.scalar_tensor_tensor(out=t[:], in0=t[:], scalar=-0.5,
                                       in1=vv[:], op0=OP.mult, op1=OP.mult)
        nc.vector.scalar_tensor_tensor(out=y[:], in0=t[:], scalar=1.5,
                                       in1=y[:], op0=OP.add, op1=OP.mult)
    return y


def _body(nc, tc, tile, mybir, bass, io):
    dt = mybir.dt
    f32, f32r, bf16 = dt.float32, dt.float32r, dt.bfloat16
    AF = mybir.ActivationFunctionType
    OP = mybir.AluOpType

    x_full, x_own, x_bf = io["x_full"], io["x_own"], io["x_bf"]
    wqkv, cb, rows, masks = io["wqkv"], io["cb"], io["rows"], io["masks"]
    wproj, w1, w2, out = io["wproj"], io["w1"], io["w2"], io["out"]

    # ---- persistent pools ----
    consts = tc.alloc_tile_pool(name="consts", bufs=1)
    persA = tc.alloc_tile_pool(name="persA", bufs=1)
    persD = tc.alloc_tile_pool(name="persD", bufs=1)
    dram = tc.alloc_tile_pool(name="dram", bufs=1, space="DRAM")

    # constants: few packed DMAs, issued on the scalar queue so that the
    # x-tile loads own the sync queue from t=0
    wqkv_sb = consts.tile([128, 8, 384], bf16, name="wqkv_sb")
    nc.scalar.dma_start(out=wqkv_sb[:],
                        in_=wqkv[:].rearrange("(cc p) d -> p cc d", p=128))
    cb_sb = consts.tile([128, 32], f32, name="cb_sb")
    nc.scalar.dma_start(out=cb_sb[:], in_=cb[:])
    rows_sb = consts.tile([1, 2944], bf16, name="rows_sb")
    nc.scalar.dma_start(out=rows_sb[:], in_=rows[:])
    mask_sb = consts.tile([128, 4, 512], bf16, name="mask_sb")
    nc.scalar.dma_start(out=mask_sb[:], in_=masks[:].rearrange("i p t -> p i t"))
    ones_b = rows_sb[0:1, 0:128]
    ones512 = rows_sb[0:1, 0:512]
    bproj_row = rows_sb[0:1, 512:1536]
    b2_row = rows_sb[0:1, 1536:2560]
    cqkv_row = rows_sb[0:1, 2560:2944]

    # phase-2 weight prefetched from t=0
    wp_sb = persD.tile([128, 8, C], bf16, name="wp_sb")
    nc.scalar.dma_start(out=wp_sb[:],
                        in_=wproj[:].rearrange("(dc p) e -> p dc e", p=128))

    # attention-persistent tensors
    qT = [persA.tile([128, T], bf16, name=f"qTb{b}") for b in range(2)]
    kT = [persA.tile([128, T], bf16, name=f"kTb{b}") for b in range(2)]
    vaug = [persA.tile([128, 16, 130], bf16, name=f"vaugb{b}") for b in range(2)]
    for b in range(2):
        nc.vector.memset(vaug[b][:, :, 64:65], 1.0)
        nc.vector.memset(vaug[b][:, :, 129:130], 1.0)

    a2a_in = [dram.tile([8, 65, TOK], bf16, name=f"a2a_in{k}") for k in range(4)]
    a2a_out = [dram.tile([8, 65, TOK], bf16, name=f"a2a_out{k}") for k in range(4)]

    w1r = w1[:].rearrange("(cc p) m -> p cc m", p=128)
    psQ = tc.alloc_tile_pool(name="psM", bufs=1, space="PSUM")

    # ======================================================================
    # Phase A: LN1 + Q/K/V per batch  (hT via DMA-xbar transpose)
    # ======================================================================
    with tc.tile_pool(name="lnq", bufs=1) as lnp:
        aT = [lnp.tile([65, T], bf16, tag="aT", bufs=2, name=f"aT{k}")
              for k in range(4)]
        for b in range(2):
            vT = lnp.tile([128, T], bf16, tag="vT", bufs=1, name=f"vT_{b}")
            with nc.named_scope(f"qkv_b{b}"):
                for tch in range(4):
                    h = lnp.tile([128, 4, C], bf16, tag="h", bufs=2,
                                 name=f"h_{b}_{tch}")
                    mvt = lnp.tile([128, 2, 4], f32, tag="mv", bufs=2,
                                   name=f"mvt_{b}_{tch}")
                    xts = []
                    for ht in range(2):
                        row0 = b * T + tch * 512 + ht * 256
                        xt = lnp.tile([128, 2, C], bf16, tag="xt", bufs=2,
                                      name=f"xt_{b}_{tch}_{ht}")
                        xts.append(xt)
                        nc.sync.dma_start(
                            out=xt[:],
                            in_=x_bf[row0:row0 + 256, :].rearrange(
                                "(s p) c -> p s c", p=128))
                        for s2 in range(2):
                            s = 2 * ht + s2
                            st = lnp.tile([128, 2, 6], f32, tag="st", bufs=2,
                                          name=f"st_{b}_{tch}_{s}")
                            nc.vector.bn_stats(out=st[:, 0, :],
                                               in_=xt[:, s2, 0:512])
                            nc.vector.bn_stats(out=st[:, 1, :],
                                               in_=xt[:, s2, 512:1024])
                            nc.vector.bn_aggr(out=mvt[:, :, s], in_=st[:])
                    # rsqrt(var+eps) for all 4 subs: quake seed + 2 Newton
                    # steps, entirely on DVE (keeps ACT on one table set)
                    rsq = _rsqrt4(nc, mybir, lnp, mvt[:, 1, :], 4,
                                  f"{b}_{tch}")
                    for ht in range(2):
                        for s2 in range(2):
                            s = 2 * ht + s2
                            nc.vector.tensor_scalar(out=h[:, s, :],
                                                    in0=xts[ht][:, s2, :],
                                                    scalar1=mvt[:, 0, s:s + 1],
                                                    scalar2=rsq[:, s:s + 1],
                                                    op0=OP.subtract, op1=OP.mult)
                    # hT chunks via xbar DMA transpose (contiguous dst per s)
                    hTb = lnp.tile([128, 4, 8, 128], bf16, tag="hTb", bufs=1,
                                   name=f"hTb_{b}_{tch}")
                    for s in range(4):
                        nc.sync.dma_start_transpose(out=hTb[:, s], in_=h[:, s, :])
                    if DEBUG and b == 0 and tch == 0:
                        nc.sync.dma_start(
                            out=io["dh"][:],
                            in_=h[:].rearrange("p a b -> p (a b)"))
                        nc.sync.dma_start(
                            out=io["dhT"][:],
                            in_=hTb[:].rearrange("p a b c -> p (a b c)"))
                    col = tch * 512
                    # q^T, k^T, v^T: N=128 matmuls (xbar dst is 128-wide);
                    # bias rows enter via a ones-row matmul; the psum->sbuf
                    # copy runs on the scalar engine (DVE is the qkv
                    # bottleneck)
                    for wi, dst in ((0, qT[b]), (1, kT[b]), (2, vT)):
                        pqk = psQ.tile([128, 512], f32, tag="pqv", bufs=2,
                                       name=f"pq_{b}_{tch}_{wi}")
                        for s in range(4):
                            for cc in range(8):
                                nc.tensor.matmul(
                                    pqk[:, s * 128:(s + 1) * 128],
                                    wqkv_sb[:, cc, wi * 128:(wi + 1) * 128],
                                    hTb[:, s, cc, :],
                                    start=(cc == 0), stop=False)
                            nc.tensor.matmul(
                                pqk[:, s * 128:(s + 1) * 128],
                                cqkv_row[0:1, wi * 128:(wi + 1) * 128],
                                ones512[0:1, 0:128],
                                start=False, stop=True)
                        nc.scalar.copy(out=dst[:, col:col + 512], in_=pqk[:])
                # v back to [t, d] layout: one full-tile xbar transpose,
                # then split into the per-head [.. 65 ..] stationary layout
                vfull = lnp.tile([128, 16, 128], bf16, tag="vfull", bufs=1,
                                 name=f"vfull_{b}")
                nc.sync.dma_start_transpose(out=vfull[:], in_=vT[:])
                nc.vector.tensor_copy(out=vaug[b][:, :, 0:64],
                                      in_=vfull[:, :, 0:64])
                nc.vector.tensor_copy(out=vaug[b][:, :, 65:129],
                                      in_=vfull[:, :, 64:128])

        # ==================================================================
        # Phase B: causal attention per (batch, local head) + its A2A
        # ==================================================================
        for b in range(2):
            for hh in range(2):
                k4 = 2 * b + hh
                hp = 64 * hh
                vs = 65 * hh
                with nc.named_scope(f"attn_b{b}h{hh}"):
                    for half in range(2):
                        qc0 = half * 1024
                        pat = [psQ.tile([128, 512], f32, tag=f"pat{i}", bufs=1,
                                        name=f"pat_{k4}_{half}_{i}")
                               for i in range(2)]
                        nsb = 8 * half + 8
                        for sb in range(nsb):
                            act0 = 0 if sb < 8 * half + 4 else 1
                            dtc = sb // 4 - 2 * half
                            ps = psQ.tile([128, 1024], f32, tag="ps", bufs=2,
                                          name=f"ps_{k4}_{half}_{sb}")
                            for i in range(act0, 2):
                                nc.tensor.matmul(
                                    ps[:, i * 512:(i + 1) * 512],
                                    kT[b][hp:hp + 64, sb * 128:sb * 128 + 128],
                                    qT[b][hp:hp + 64,
                                          qc0 + i * 512:qc0 + (i + 1) * 512],
                                    start=True, stop=True)
                            pt = lnp.tile([128, 1024], bf16, tag="pt", bufs=3,
                                          name=f"pt_{k4}_{half}_{sb}")
                            nc.scalar.activation(out=pt[:, act0 * 512:1024],
                                                 in_=ps[:, act0 * 512:1024],
                                                 func=AF.Exp, scale=0.125)
                            if dtc >= act0:
                                nc.vector.tensor_mul(
                                    pt[:, dtc * 512:(dtc + 1) * 512],
                                    pt[:, dtc * 512:(dtc + 1) * 512],
                                    mask_sb[:, sb % 4, :])
                            for i in range(act0, 2):
                                last = 8 * half + 3 if i == 0 else nsb - 1
                                nc.tensor.matmul(
                                    pat[i][0:65, :], vaug[b][:, sb, vs:vs + 65],
                                    pt[:, i * 512:(i + 1) * 512],
                                    start=(sb == 0), stop=(sb == last))
                        for i in range(2):
                            qcol = (2 * half + i) * 512
                            nc.vector.tensor_copy(
                                out=aT[k4][64:65, qcol:qcol + 512],
                                in_=pat[i][64:65, :])
                            nc.vector.tensor_copy(
                                out=aT[k4][0:64, qcol:qcol + 512],
                                in_=pat[i][0:64, :])
                if DEBUG and k4 == 0:
                    nc.sync.dma_start(out=io["dq"][:], in_=qT[0][:])
                    nc.sync.dma_start(out=io["dk"][:], in_=kT[0][:])
                    nc.sync.dma_start(
                        out=io["dva"][:],
                        in_=vaug[0][:].rearrange("p a b -> p (a b)"))
                    nc.sync.dma_start(out=io["da"][:], in_=aT[0][:])
                # ship: shard j = tokens [256j, 256j+256) of this (b, head)
                nc.sync.dma_start(
                    out=a2a_in[k4][:].rearrange("j d t -> d j t"),
                    in_=aT[k4][:].rearrange("d (j t) -> d j t", j=8))
                nc.gpsimd.collective_compute(
                    "AllToAll", mybir.AluOpType.bypass,
                    replica_groups=[list(range(NCORES))],
                    ins=[a2a_in[k4][:].opt()], outs=[a2a_out[k4][:].opt()])
                if DEBUG and k4 == 0:
                    nc.sync.dma_start(out=io["din"][:], in_=a2a_in[0][:])
                    nc.sync.dma_start(out=io["dout"][:], in_=a2a_out[0][:])

    # ======================================================================
    # Phase C: per batch: unpack A2A + normalize + proj + LN2 + FFN1
    # ======================================================================
    x2 = persD.tile([128, 4, C], f32, name="x2")
    ff1T = persD.tile([128, 32, 512], bf16, name="ff1T")
    h2T = [persD.tile([128, 2, 8, 128], bf16, name=f"h2T_{b}") for b in range(2)]

    for b in range(2):
        tcol = TOK * b
        with tc.tile_pool(name=f"prj{b}", bufs=1) as prp:
            xo = prp.tile([128, 2, C], f32, tag="xo", name=f"xo_{b}")
            nc.gpsimd.dma_start(
                out=xo[:],
                in_=x_own[TOK * b:TOK * (b + 1), :].rearrange(
                    "(q p) c -> p q c", p=128))
            aT_own = prp.tile([128, 8, TOK], bf16, tag="aTo", name=f"aTo_{b}")
            h2p = prp.tile([128, 8, TOK], bf16, tag="h2p", name=f"h2p_{b}")
            rb = prp.tile([128, 8, TOK], bf16, tag="rb", name=f"rb_{b}")
            for hh in range(2):
                k4 = 2 * b + hh
                nc.sync.dma_start(
                    out=aT_own[64 * hh:64 * hh + 64, :, :],
                    in_=a2a_out[k4][:, 0:64, :].rearrange("r d t -> d r t"))
                for r in range(8):
                    nc.sync.dma_start(
                        out=rb[64 * hh:64 * hh + 64, r, :],
                        in_=a2a_out[k4][r, 64:65, :].to_broadcast([64, TOK]))
            for r in range(8):
                rbf = prp.tile([128, TOK], f32, tag="rbf", bufs=2,
                               name=f"rbf_{b}_{r}")
                nc.vector.tensor_copy(out=rbf[:], in_=rb[:, r, :])
                rbf2 = prp.tile([128, TOK], f32, tag="rbf2", bufs=2,
                                name=f"rbf2_{b}_{r}")
                nc.vector.reciprocal_approx_fast(out=rbf2[:], in_=rbf[:])
                nc.vector.tensor_mul(aT_own[:, r, :], aT_own[:, r, :],
                                     rbf2[:])
            if DEBUG and b == 0:
                nc.sync.dma_start(
                    out=io["dao"][:],
                    in_=aT_own[:].rearrange("p a b -> p (a b)"))
            h2 = prp.tile([128, 2, C], bf16, tag="h2", name=f"h2_{b}")
            with nc.named_scope(f"proj_ln2_b{b}"):
                mv2t = prp.tile([128, 2, 2], f32, tag="mv2", bufs=2,
                                name=f"mv2_{b}")
                for tqi in range(2):
                    tq = 2 * b + tqi
                    for eh in range(2):
                        pp = psQ.tile([128, 512], f32, tag="pqv", bufs=2,
                                      name=f"pp_{tq}_{eh}")
                        for dc in range(8):
                            nc.tensor.matmul(
                                pp[:], aT_own[:, dc, tqi * 128:(tqi + 1) * 128],
                                wp_sb[:, dc, eh * 512:eh * 512 + 512],
                                start=(dc == 0), stop=False)
                        nc.tensor.matmul(pp[:], ones_b,
                                         bproj_row[0:1, eh * 512:eh * 512 + 512],
                                         start=False, stop=True)
                        nc.vector.tensor_add(x2[:, tq, eh * 512:eh * 512 + 512],
                                             pp[:], xo[:, tqi, eh * 512:eh * 512 + 512])
                    st2 = prp.tile([128, 2, 6], f32, tag="st2", bufs=2,
                                   name=f"st2_{tq}")
                    nc.vector.bn_stats(out=st2[:, 0, :], in_=x2[:, tq, 0:512])
                    nc.vector.bn_stats(out=st2[:, 1, :], in_=x2[:, tq, 512:1024])
                    nc.vector.bn_aggr(out=mv2t[:, :, tqi], in_=st2[:])
                rsq2 = _rsqrt4(nc, mybir, prp, mv2t[:, 1, :], 2, f"ln2_{b}")
                for tqi in range(2):
                    tq = 2 * b + tqi
                    nc.vector.tensor_scalar(out=h2[:, tqi, :], in0=x2[:, tq, :],
                                            scalar1=mv2t[:, 0, tqi:tqi + 1],
                                            scalar2=rsq2[:, tqi:tqi + 1],
                                            op0=OP.subtract, op1=OP.mult)
                    nc.sync.dma_start_transpose(out=h2T[b][:, tqi],
                                                in_=h2[:, tqi, :])
                # repack [tq, cc, 128] -> [cc, 256] (contiguous moving
                # operand for FFN1)
                for tqi in range(2):
                    nc.vector.tensor_copy(
                        out=h2p[:, :, tqi * 128:(tqi + 1) * 128],
                        in_=h2T[b][:, tqi])
            with nc.named_scope(f"ffn1_b{b}"):
                for w in range(16):
                    w1w = prp.tile([128, 8, 256], bf16, tag="w1w", bufs=3,
                                   name=f"w1w_{b}_{w}")
                    nc.gpsimd.dma_start(out=w1w[:],
                                        in_=w1r[:, :, w * 256:(w + 1) * 256])
                    for m2 in range(2):
                        m = w * 2 + m2
                        pf = psQ.tile([128, 512], f32, tag="pqv", bufs=2,
                                      name=f"pf_{b}_{m}")
                        for cc in range(8):
                            nc.tensor.matmul(
                                pf[:, 0:256], w1w[:, cc, m2 * 128:(m2 + 1) * 128],
                                h2p[:, cc, :],
                                start=(cc == 0), stop=(cc == 7))
                        nc.scalar.activation(out=ff1T[:, m, tcol:tcol + TOK],
                                             in_=pf[:, 0:256], func=AF.Relu,
                                             bias=cb_sb[:, m:m + 1])
            if DEBUG and b == 0:
                nc.sync.dma_start(
                    out=io["dh2p"][:],
                    in_=h2p[:].rearrange("p a b -> p (a b)"))
    if DEBUG:
        nc.sync.dma_start(out=io["dx2"][:],
                          in_=x2[:].rearrange("p a b -> p (a b)"))
        nc.sync.dma_start(out=io["df1"][:],
                          in_=ff1T[:].rearrange("p a b -> p (a b)"))

    psQ.release()

    # ======================================================================
    # Phase D: FFN2 over all 4 token-quarters, single w2 stream
    # ======================================================================
    with tc.tile_pool(name="ffn2p", bufs=1) as f2p, \
         tc.tile_pool(name="ps2", bufs=1, space="PSUM") as ps2p, \
         nc.named_scope("ffn2"):
        pso = [ps2p.tile([128, C], f32, tag="pso", bufs=4, name=f"pso_{tq}")
               for tq in range(4)]
        for mc in range(32):
            w2t = f2p.tile([128, C], bf16, tag="w2t", bufs=6, name=f"w2t_{mc}")
            nc.gpsimd.dma_start(out=w2t[:], in_=w2[mc * 128:(mc + 1) * 128, :])
            for tq in range(4):
                for eh in range(2):
                    nc.tensor.matmul(pso[tq][:, eh * 512:(eh + 1) * 512],
                                     ff1T[:, mc, tq * 128:(tq + 1) * 128],
                                     w2t[:, eh * 512:(eh + 1) * 512],
                                     start=(mc == 0), stop=False)
        for tq in range(4):
            for eh in range(2):
                nc.tensor.matmul(pso[tq][:, eh * 512:(eh + 1) * 512],
                                 ones_b, b2_row[0:1, eh * 512:(eh + 1) * 512],
                                 start=False, stop=True)
            ot = f2p.tile([128, C], f32, tag="ot", bufs=2, name=f"ot_{tq}")
            nc.vector.tensor_add(ot[:], pso[tq][:], x2[:, tq, :])
            nc.gpsimd.dma_start(out=out[tq * 128:(tq + 1) * 128, :], in_=ot[:])
    persD.release()
    persA.release()
    consts.release()
    dram.release()


# --------------------------------------------------------------------------
# host driver
# --------------------------------------------------------------------------
def _make_in_maps(inputs):
    bf = ml_dtypes.bfloat16
    x = np.ascontiguousarray(np.asarray(inputs["x"], np.float32))
    wq = np.asarray(inputs["wq"], np.float32)
    wk = np.asarray(inputs["wk"], np.float32)
    wv = np.asarray(inputs["wv"], np.float32)
    w_proj = np.asarray(inputs["w_proj"], np.float32)
    b_proj = np.asarray(inputs["b_proj"], np.float32)
    w1 = np.asarray(inputs["w1"], np.float32)
    b1 = np.asarray(inputs["b1"], np.float32)
    w2 = np.asarray(inputs["w2"], np.float32)
    b2 = np.asarray(inputs["b2"], np.float32)
    g1 = np.asarray(inputs["g1"], np.float32)
    be1 = np.asarray(inputs["be1"], np.float32)
    g2 = np.asarray(inputs["g2"], np.float32)
    be2 = np.asarray(inputs["be2"], np.float32)

    xf = x.reshape(BT, C)
    i_mask = np.zeros((4, 128, 512), np.float32)
    s_idx = np.arange(128)[:, None]
    t_idx = np.arange(512)[None, :]
    for i in range(4):
        i_mask[i] = (s_idx + 128 * i <= t_idx).astype(np.float32)

    w1f = g2[:, None] * w1                       # fold LN2 gamma
    b1f = b1 + be2 @ w1                          # fold LN2 beta

    common = dict(
        x_full=xf,
        x_bf=xf.astype(bf),
        masks=i_mask.astype(bf),
        cb=np.ascontiguousarray(b1f.reshape(32, 128).T),
        wproj=np.ascontiguousarray(w_proj).astype(bf),
        w1=np.ascontiguousarray(w1f).astype(bf),
        w2=np.ascontiguousarray(w2).astype(bf),
    )
    in_maps = []
    for c in range(NCORES):
        wqp = np.concatenate([wq[2 * c], wq[2 * c + 1]], axis=1)  # [C,128]
        wkp = np.concatenate([wk[2 * c], wk[2 * c + 1]], axis=1)
        wvp = np.concatenate([wv[2 * c], wv[2 * c + 1]], axis=1)
        wqkv = np.concatenate([g1[:, None] * wqp, g1[:, None] * wkp,
                               g1[:, None] * wvp], axis=1)        # [C,384]
        rows = np.concatenate([np.ones(512, np.float32), b_proj, b2,
                               be1 @ wqp, be1 @ wkp, be1 @ wvp])[None, :]
        m = dict(common)
        m["wqkv"] = np.ascontiguousarray(wqkv).astype(bf)
        m["rows"] = np.ascontiguousarray(rows).astype(bf)
        m["x_own"] = np.ascontiguousarray(
            np.concatenate([xf[TOK * c:TOK * (c + 1)],
                            xf[T + TOK * c:T + TOK * (c + 1)]], axis=0))
        in_maps.append(m)
    return in_maps


LAST_RESULTS = None


def kernel(trace=False, **inputs):
    global LAST_RESULTS
    from concourse import bass_utils

    if "nc" not in _CACHE:
        _CACHE["nc"] = _build_program()
    nc = _CACHE["nc"]
    in_maps = _make_in_maps(inputs)
    res = bass_utils.run_bass_kernel_spmd(
        nc, in_maps, core_ids=list(range(NCORES)), trace=trace)
    LAST_RESULTS = res
    out = np.zeros((B, T, C), np.float32)
    for c in range(NCORES):
        r = res.results[c]["out"]
        out[0, TOK * c:TOK * (c + 1), :] = r[0:TOK]
        out[1, TOK * c:TOK * (c + 1), :] = r[TOK:2 * TOK]
    return out


# revision 45
# speedup vs baseline: 1.1197x; 1.0227x over previous
"""Trainium2 Bass kernel for a dense pre-LN transformer block.

Problem: B=2, T=2048, C=1024, H=16 heads (d=64), FFN 4x, causal attention.

Parallelization over 8 NeuronCores (single SPMD program, one launch):
  - Attention: head-tensor-parallel. Core c computes heads {2c, 2c+1} for
    both batches: LN1 (replicated), Q/K/V projections, causal-block
    attention with unnormalized softmax (denominator via a ones-column in
    the value tile), reciprocal of the denominator computed sender-side.
  - FOUR AllToAlls (one per (batch, local-head)) redistribute attn^T from
    head-split to token-split; each overlaps the next attention unit or
    the early FFN work, so almost no collective time is exposed.
  - Post-A2A: core c owns tokens [256c, 256c+256) of BOTH batches:
    output projection + residual, LN2, FFN, residual.

Key implementation choices:
  - All [t,c] -> [c,t] transposes run on the DMA engines (xbar
    dma_start_transpose, bf16) instead of the PE: frees ~100us of PE time.
  - LayerNorm rsqrt = exp(-0.5*log(var+eps)) on the scalar engine so the
    whole kernel uses one activation table set (no table reload thrash).
  - g/beta of both LayerNorms are folded into the adjacent weight
    matrices host-side (bias rows enter via ones-row matmuls).
  - Softmax normalization: sender computes recip(den) (single-pass
    approx), the reciprocal rides the A2A as row 64; receiver applies it
    with one broadcast-DMA + multiply per batch (no expensive DVE
    reciprocal on broadcast data).
  - Causal masks multiply on GpSimd (otherwise idle), constants arrive in
    a handful of packed DMAs issued on the scalar queue so the x-tile DMAs
    lead the sync queue.
"""

import numpy as np
import ml_dtypes

B, T, C = 2, 2048, 1024
H, D = 16, 64
FF = 4 * C
EPS = 1e-5
NCORES = 8
TOK = 256   # tokens owned per core PER BATCH in the post-A2A phase
BT = B * T

_CACHE = {}
DEBUG = False


# --------------------------------------------------------------------------
# device program
# --------------------------------------------------------------------------
def _build_program():
    import concourse.bass as bass
    import concourse.mybir as mybir
    import concourse.tile as tile
    from concourse import bacc

    dt = mybir.dt
    f32 = dt.float32

    nc = bacc.Bacc("TRN2", target_bir_lowering=False, debug=False,
                   num_devices=NCORES)

    bf16 = dt.bfloat16
    x_full = nc.dram_tensor("x_full", [BT, C], f32, kind="ExternalInput")
    x_bf = nc.dram_tensor("x_bf", [BT, C], bf16, kind="ExternalInput")
    x_own = nc.dram_tensor("x_own", [2 * TOK, C], f32, kind="ExternalInput")
    wqkv = nc.dram_tensor("wqkv", [C, 384], bf16, kind="ExternalInput")
    cb = nc.dram_tensor("cb", [128, 32], f32, kind="ExternalInput")
    rows = nc.dram_tensor("rows", [1, 2944], bf16, kind="ExternalInput")
    masks = nc.dram_tensor("masks", [4, 128, 512], bf16, kind="ExternalInput")
    wproj = nc.dram_tensor("wproj", [C, C], bf16, kind="ExternalInput")
    w1 = nc.dram_tensor("w1", [C, FF], bf16, kind="ExternalInput")
    w2 = nc.dram_tensor("w2", [FF, C], bf16, kind="ExternalInput")
    out = nc.dram_tensor("out", [2 * TOK, C], f32, kind="ExternalOutput")
    if DEBUG:
        dh = nc.dram_tensor("dh", [128, 4 * C], bf16, kind="ExternalOutput")
        dhT = nc.dram_tensor("dhT", [128, 4 * 8 * 128], bf16,
                             kind="ExternalOutput")
        dq = nc.dram_tensor("dq", [128, T], bf16, kind="ExternalOutput")
        dk = nc.dram_tensor("dk", [128, T], bf16, kind="ExternalOutput")
        dva = nc.dram_tensor("dva", [128, 16 * 130], bf16, kind="ExternalOutput")
        da = nc.dram_tensor("da", [65, T], bf16, kind="ExternalOutput")
        din = nc.dram_tensor("din", [8, 65, TOK], bf16, kind="ExternalOutput")
        dout = nc.dram_tensor("dout", [8, 65, TOK], bf16, kind="ExternalOutput")
        dao = nc.dram_tensor("dao", [128, 8 * TOK], bf16, kind="ExternalOutput")
        dx2 = nc.dram_tensor("dx2", [128, 4 * C], f32, kind="ExternalOutput")
        dh2p = nc.dram_tensor("dh2p", [128, 8 * TOK], bf16, kind="ExternalOutput")
        df1 = nc.dram_tensor("df1", [128, 32 * 512], bf16, kind="ExternalOutput")

    with tile.TileContext(nc, num_cores=NCORES) as tc:
        _body(nc, tc, tile, mybir, bass, locals())
    nc.compile()
    return nc


def _rsqrt4(nc, mybir, pool, var_ap, n, name):
    """rsqrt(var + EPS) on DVE: quake bit-trick seed + 2 Newton steps.

    var_ap: [128, n] f32 (may be strided). Returns a [128, n] f32 tile.
    Avoids the scalar engine entirely so the activation table never leaves
    the exp set."""
    dt = mybir.dt
    f32, i32, u32 = dt.float32, dt.int32, dt.uint32
    OP = mybir.AluOpType
    vv = pool.tile([128, n], f32, tag="vv", bufs=2, name=f"vv_{name}")
    nc.vector.tensor_scalar_add(out=vv[:], in0=var_ap, scalar1=EPS)
    y = pool.tile([128, n], f32, tag="yy", bufs=2, name=f"yy_{name}")
    nc.vector.tensor_scalar(out=y[:].bitcast(u32), in0=vv[:].bitcast(u32),
                            scalar1=1, scalar2=None,
                            op0=OP.logical_shift_right)
    nc.vector.tensor_scalar(out=y[:].bitcast(i32), in0=y[:].bitcast(i32),
                            scalar1=0x5F3759DF, scalar2=-1,
                            op0=OP.subtract, op1=OP.mult)
    t = pool.tile([128, n], f32, tag="tt", bufs=2, name=f"tt_{name}")
    for _ in range(2):
        nc.vector.tensor_mul(t[:], y[:], y[:])
        nc.vector.scalar_tensor_tensor(out=t[:], in0=t[:], scalar=-0.5,
                                       in1=vv[:], op0=OP.mult, op1=OP.mult)
        nc.vector.scalar_tensor_tensor(out=y[:], in0=t[:], scalar=1.5,
                                       in1=y[:], op0=OP.add, op1=OP.mult)
    return y


def _body(nc, tc, tile, mybir, bass, io):
    dt = mybir.dt
    f32, f32r, bf16 = dt.float32, dt.float32r, dt.bfloat16
    AF = mybir.ActivationFunctionType
    OP = mybir.AluOpType

    x_full, x_own, x_bf = io["x_full"], io["x_own"], io["x_bf"]
    wqkv, cb, rows, masks = io["wqkv"], io["cb"], io["rows"], io["masks"]
    wproj, w1, w2, out = io["wproj"], io["w1"], io["w2"], io["out"]

    # ---- persistent pools ----
    consts = tc.alloc_tile_pool(name="consts", bufs=1)
    persA = tc.alloc_tile_pool(name="persA", bufs=1)
    persD = tc.alloc_tile_pool(name="persD", bufs=1)
    dram = tc.alloc_tile_pool(name="dram", bufs=1, space="DRAM")

    # constants: few packed DMAs, issued on the scalar queue so that the
    # x-tile loads own the sync queue from t=0
    wqkv_sb = consts.tile([128, 8, 384], bf16, name="wqkv_sb")
    nc.scalar.dma_start(out=wqkv_sb[:],
                        in_=wqkv[:].rearrange("(cc p) d -> p cc d", p=128))
    cb_sb = consts.tile([128, 32], f32, name="cb_sb")
    nc.scalar.dma_start(out=cb_sb[:], in_=cb[:])
    rows_sb = consts.tile([1, 2944], bf16, name="rows_sb")
    nc.scalar.dma_start(out=rows_sb[:], in_=rows[:])
    mask_sb = consts.tile([128, 4, 512], bf16, name="mask_sb")
    nc.scalar.dma_start(out=mask_sb[:], in_=masks[:].rearrange("i p t -> p i t"))
    ones_b = rows_sb[0:1, 0:128]
    ones512 = rows_sb[0:1, 0:512]
    bproj_row = rows_sb[0:1, 512:1536]
    b2_row = rows_sb[0:1, 1536:2560]
    cqkv_row = rows_sb[0:1, 2560:2944]

    # phase-2 weight prefetched from t=0
    wp_sb = persD.tile([128, 8, C], bf16, name="wp_sb")
    nc.scalar.dma_start(out=wp_sb[:],
                        in_=wproj[:].rearrange("(dc p) e -> p dc e", p=128))

    # attention-persistent tensors
    qT = [persA.tile([128, T], bf16, name=f"qTb{b}") for b in range(2)]
    kT = [persA.tile([128, T], bf16, name=f"kTb{b}") for b in range(2)]
    vaug = [persA.tile([128, 16, 130], bf16, name=f"vaugb{b}") for b in range(2)]
    for b in range(2):
        nc.vector.memset(vaug[b][:, :, 64:65], 1.0)
        nc.vector.memset(vaug[b][:, :, 129:130], 1.0)

    a2a_in = [dram.tile([8, 65, TOK], bf16, name=f"a2a_in{k}") for k in range(4)]
    a2a_out = [dram.tile([8, 65, TOK], bf16, name=f"a2a_out{k}") for k in range(4)]

    w1r = w1[:].rearrange("(cc p) m -> p cc m", p=128)
    psQ = tc.alloc_tile_pool(name="psM", bufs=1, space="PSUM")

    # ======================================================================
    # Phase A: LN1 + Q/K/V per batch  (hT via DMA-xbar transpose)
    # ======================================================================
    with tc.tile_pool(name="lnq", bufs=1) as lnp:
        aT = [lnp.tile([65, T], bf16, tag="aT", bufs=2, name=f"aT{k}")
              for k in range(4)]
        for b in range(2):
            vT = lnp.tile([128, T], bf16, tag="vT", bufs=1, name=f"vT_{b}")
            with nc.named_scope(f"qkv_b{b}"):
                for tch in range(4):
                    h = lnp.tile([128, 4, C], bf16, tag="h", bufs=2,
                                 name=f"h_{b}_{tch}")
                    mvt = lnp.tile([128, 2, 4], f32, tag="mv", bufs=2,
                                   name=f"mvt_{b}_{tch}")
                    xts = []
                    for ht in range(2):
                        row0 = b * T + tch * 512 + ht * 256
                        xt = lnp.tile([128, 2, C], bf16, tag="xt", bufs=2,
                                      name=f"xt_{b}_{tch}_{ht}")
                        xts.append(xt)
                        nc.sync.dma_start(
                            out=xt[:],
                            in_=x_bf[row0:row0 + 256, :].rearrange(
                                "(s p) c -> p s c", p=128))
                        for s2 in range(2):
                            s = 2 * ht + s2
                            st = lnp.tile([128, 2, 6], f32, tag="st", bufs=2,
                                          name=f"st_{b}_{tch}_{s}")
                            nc.vector.bn_stats(out=st[:, 0, :],
                                               in_=xt[:, s2, 0:512])
                            nc.vector.bn_stats(out=st[:, 1, :],
                                               in_=xt[:, s2, 512:1024])
                            nc.vector.bn_aggr(out=mvt[:, :, s], in_=st[:])
                    # rsqrt(var+eps) for all 4 subs: quake seed + 2 Newton
                    # steps, entirely on DVE (keeps ACT on one table set)
                    rsq = _rsqrt4(nc, mybir, lnp, mvt[:, 1, :], 4,
                                  f"{b}_{tch}")
                    for ht in range(2):
                        for s2 in range(2):
                            s = 2 * ht + s2
                            nc.vector.tensor_scalar(out=h[:, s, :],
                                                    in0=xts[ht][:, s2, :],
                                                    scalar1=mvt[:, 0, s:s + 1],
                                                    scalar2=rsq[:, s:s + 1],
                                                    op0=OP.subtract, op1=OP.mult)
                    # hT chunks via xbar DMA transpose (contiguous dst per s)
                    hTb = lnp.tile([128, 4, 8, 128], bf16, tag="hTb", bufs=1,
                                   name=f"hTb_{b}_{tch}")
                    for s in range(4):
                        nc.sync.dma_start_transpose(out=hTb[:, s], in_=h[:, s, :])
                    if DEBUG and b == 0 and tch == 0:
                        nc.sync.dma_start(
                            out=io["dh"][:],
                            in_=h[:].rearrange("p a b -> p (a b)"))
                        nc.sync.dma_start(
                            out=io["dhT"][:],
                            in_=hTb[:].rearrange("p a b c -> p (a b c)"))
                    col = tch * 512
                    # q^T, k^T, v^T: N=128 matmuls (xbar dst is 128-wide);
                    # bias rows enter via a ones-row matmul; the psum->sbuf
                    # copy runs on the scalar engine (DVE is the qkv
                    # bottleneck)
                    for wi, dst in ((0, qT[b]), (1, kT[b]), (2, vT)):
                        pqk = psQ.tile([128, 512], f32, tag="pqv", bufs=2,
                                       name=f"pq_{b}_{tch}_{wi}")
                        for s in range(4):
                            for cc in range(8):
                                nc.tensor.matmul(
                                    pqk[:, s * 128:(s + 1) * 128],
                                    wqkv_sb[:, cc, wi * 128:(wi + 1) * 128],
                                    hTb[:, s, cc, :],
                                    start=(cc == 0), stop=False)
                            nc.tensor.matmul(
                                pqk[:, s * 128:(s + 1) * 128],
                                cqkv_row[0:1, wi * 128:(wi + 1) * 128],
                                ones512[0:1, 0:128],
                                start=False, stop=True)
                        nc.scalar.copy(out=dst[:, col:col + 512], in_=pqk[:])
                # v back to [t, d] layout: one full-tile xbar transpose,
                # then split into the per-head [.. 65 ..] stationary layout
                vfull = lnp.tile([128, 16, 128], bf16, tag="vfull", bufs=1,
                                 name=f"vfull_{b}")
                nc.sync.dma_start_transpose(out=vfull[:], in_=vT[:])
                nc.vector.tensor_copy(out=vaug[b][:, :, 0:64],
                                      in_=vfull[:, :, 0:64])
                nc.vector.tensor_copy(out=vaug[b][:, :, 65:129],
                                      in_=vfull[:, :, 64:128])

        # ==================================================================
        # Phase B: causal attention per (batch, local head) + its A2A
        # ==================================================================
        for b in range(2):
            for hh in range(2):
                k4 = 2 * b + hh
                hp = 64 * hh
                vs = 65 * hh
                with nc.named_scope(f"attn_b{b}h{hh}"):
                    for half in range(2):
                        qc0 = half * 1024
                        pat = [psQ.tile([128, 512], f32, tag=f"pat{i}", bufs=1,
                                        name=f"pat_{k4}_{half}_{i}")
                               for i in range(2)]
                        nsb = 8 * half + 8
                        for sb in range(nsb):
                            act0 = 0 if sb < 8 * half + 4 else 1
                            dtc = sb // 4 - 2 * half
                            ps = psQ.tile([128, 1024], f32, tag="ps", bufs=2,
                                          name=f"ps_{k4}_{half}_{sb}")
                            for i in range(act0, 2):
                                nc.tensor.matmul(
                                    ps[:, i * 512:(i + 1) * 512],
                                    kT[b][hp:hp + 64, sb * 128:sb * 128 + 128],
                                    qT[b][hp:hp + 64,
                                          qc0 + i * 512:qc0 + (i + 1) * 512],
                                    start=True, stop=True)
                            pt = lnp.tile([128, 1024], bf16, tag="pt", bufs=3,
                                          name=f"pt_{k4}_{half}_{sb}")
                            nc.scalar.activation(out=pt[:, act0 * 512:1024],
                                                 in_=ps[:, act0 * 512:1024],
                                                 func=AF.Exp, scale=0.125)
                            if dtc >= act0:
                                nc.vector.tensor_mul(
                                    pt[:, dtc * 512:(dtc + 1) * 512],
                                    pt[:, dtc * 512:(dtc + 1) * 512],
                                    mask_sb[:, sb % 4, :])
                            for i in range(act0, 2):
                                last = 8 * half + 3 if i == 0 else nsb - 1
                                nc.tensor.matmul(
                                    pat[i][0:65, :], vaug[b][:, sb, vs:vs + 65],
                                    pt[:, i * 512:(i + 1) * 512],
                                    start=(sb == 0), stop=(sb == last))
                        for i in range(2):
                            qcol = (2 * half + i) * 512
                            nc.vector.tensor_copy(
                                out=aT[k4][64:65, qcol:qcol + 512],
                                in_=pat[i][64:65, :])
                            nc.vector.tensor_copy(
                                out=aT[k4][0:64, qcol:qcol + 512],
                                in_=pat[i][0:64, :])
                if DEBUG and k4 == 0:
                    nc.sync.dma_start(out=io["dq"][:], in_=qT[0][:])
                    nc.sync.dma_start(out=io["dk"][:], in_=kT[0][:])
                    nc.sync.dma_start(
                        out=io["dva"][:],
                        in_=vaug[0][:].rearrange("p a b -> p (a b)"))
                    nc.sync.dma_start(out=io["da"][:], in_=aT[0][:])
                # ship: shard j = tokens [256j, 256j+256) of this (b, head)
                nc.sync.dma_start(
                    out=a2a_in[k4][:].rearrange("j d t -> d j t"),
                    in_=aT[k4][:].rearrange("d (j t) -> d j t", j=8))
                nc.gpsimd.collective_compute(
                    "AllToAll", mybir.AluOpType.bypass,
                    replica_groups=[list(range(NCORES))],
                    ins=[a2a_in[k4][:].opt()], outs=[a2a_out[k4][:].opt()])
                if DEBUG and k4 == 0:
                    nc.sync.dma_start(out=io["din"][:], in_=a2a_in[0][:])
                    nc.sync.dma_start(out=io["dout"][:], in_=a2a_out[0][:])

    # ======================================================================
    # Phase C: per batch: unpack A2A + normalize + proj + LN2 + FFN1
    # ======================================================================
    x2 = persD.tile([128, 4, C], f32, name="x2")
    ff1T = persD.tile([128, 32, 512], bf16, name="ff1T")
    h2T = [persD.tile([128, 2, 8, 128], bf16, name=f"h2T_{b}") for b in range(2)]

    for b in range(2):
        tcol = TOK * b
        with tc.tile_pool(name=f"prj{b}", bufs=1) as prp:
            xo = prp.tile([128, 2, C], f32, tag="xo", name=f"xo_{b}")
            nc.gpsimd.dma_start(
                out=xo[:],
                in_=x_own[TOK * b:TOK * (b + 1), :].rearrange(
                    "(q p) c -> p q c", p=128))
            aT_own = prp.tile([128, 8, TOK], bf16, tag="aTo", name=f"aTo_{b}")
            h2p = prp.tile([128, 8, TOK], bf16, tag="h2p", name=f"h2p_{b}")
            rb = prp.tile([128, 8, TOK], bf16, tag="rb", name=f"rb_{b}")
            for hh in range(2):
                k4 = 2 * b + hh
                nc.gpsimd.dma_start(
                    out=aT_own[64 * hh:64 * hh + 64, :, :],
                    in_=a2a_out[k4][:, 0:64, :].rearrange("r d t -> d r t"))
                for r in range(8):
                    nc.gpsimd.dma_start(
                        out=rb[64 * hh:64 * hh + 64, r, :],
                        in_=a2a_out[k4][r, 64:65, :].to_broadcast([64, TOK]))
            for r in range(8):
                rbf = prp.tile([128, TOK], f32, tag="rbf", bufs=2,
                               name=f"rbf_{b}_{r}")
                nc.vector.tensor_copy(out=rbf[:], in_=rb[:, r, :])
                rbf2 = prp.tile([128, TOK], f32, tag="rbf2", bufs=2,
                                name=f"rbf2_{b}_{r}")
                nc.vector.reciprocal_approx_fast(out=rbf2[:], in_=rbf[:])
                nc.vector.tensor_mul(aT_own[:, r, :], aT_own[:, r, :],
                                     rbf2[:])
            if DEBUG and b == 0:
                nc.sync.dma_start(
                    out=io["dao"][:],
                    in_=aT_own[:].rearrange("p a b -> p (a b)"))
            h2 = prp.tile([128, 2, C], bf16, tag="h2", name=f"h2_{b}")
            with nc.named_scope(f"proj_ln2_b{b}"):
                mv2t = prp.tile([128, 2, 2], f32, tag="mv2", bufs=2,
                                name=f"mv2_{b}")
                for tqi in range(2):
                    tq = 2 * b + tqi
                    for eh in range(2):
                        pp = psQ.tile([128, 512], f32, tag="pqv", bufs=2,
                                      name=f"pp_{tq}_{eh}")
                        for dc in range(8):
                            nc.tensor.matmul(
                                pp[:], aT_own[:, dc, tqi * 128:(tqi + 1) * 128],
                                wp_sb[:, dc, eh * 512:eh * 512 + 512],
                                start=(dc == 0), stop=False)
                        nc.tensor.matmul(pp[:], ones_b,
                                         bproj_row[0:1, eh * 512:eh * 512 + 512],
                                         start=False, stop=True)
                        nc.vector.tensor_add(x2[:, tq, eh * 512:eh * 512 + 512],
                                             pp[:], xo[:, tqi, eh * 512:eh * 512 + 512])
                    st2 = prp.tile([128, 2, 6], f32, tag="st2", bufs=2,
                                   name=f"st2_{tq}")
                    nc.vector.bn_stats(out=st2[:, 0, :], in_=x2[:, tq, 0:512])
                    nc.vector.bn_stats(out=st2[:, 1, :], in_=x2[:, tq, 512:1024])
                    nc.vector.bn_aggr(out=mv2t[:, :, tqi], in_=st2[:])
                rsq2 = _rsqrt4(nc, mybir, prp, mv2t[:, 1, :], 2, f"ln2_{b}")
                for tqi in range(2):
                    tq = 2 * b + tqi
                    nc.vector.tensor_scalar(out=h2[:, tqi, :], in0=x2[:, tq, :],
                                            scalar1=mv2t[:, 0, tqi:tqi + 1],
                                            scalar2=rsq2[:, tqi:tqi + 1],
                                            op0=OP.subtract, op1=OP.mult)
                    nc.sync.dma_start_transpose(out=h2T[b][:, tqi],
                                                in_=h2[:, tqi, :])
                # repack [tq, cc, 128] -> [cc, 256] (contiguous moving
                # operand for FFN1)
                for tqi in range(2):
                    nc.vector.tensor_copy(
                        out=h2p[:, :, tqi * 128:(tqi + 1) * 128],
                        in_=h2T[b][:, tqi])
            with nc.named_scope(f"ffn1_b{b}"):
                for w in range(16):
                    w1w = prp.tile([128, 8, 256], bf16, tag="w1w", bufs=3,
                                   name=f"w1w_{b}_{w}")
                    nc.gpsimd.dma_start(out=w1w[:],
                                        in_=w1r[:, :, w * 256:(w + 1) * 256])
                    for m2 in range(2):
                        m = w * 2 + m2
                        pf = psQ.tile([128, 512], f32, tag="pqv", bufs=2,
                                      name=f"pf_{b}_{m}")
                        for cc in range(8):
                            nc.tensor.matmul(
                                pf[:, 0:256], w1w[:, cc, m2 * 128:(m2 + 1) * 128],
                                h2p[:, cc, :],
                                start=(cc == 0), stop=(cc == 7))
                        nc.scalar.activation(out=ff1T[:, m, tcol:tcol + TOK],
                                             in_=pf[:, 0:256], func=AF.Relu,
                                             bias=cb_sb[:, m:m + 1])
            if DEBUG and b == 0:
                nc.sync.dma_start(
                    out=io["dh2p"][:],
                    in_=h2p[:].rearrange("p a b -> p (a b)"))
    if DEBUG:
        nc.sync.dma_start(out=io["dx2"][:],
                          in_=x2[:].rearrange("p a b -> p (a b)"))
        nc.sync.dma_start(out=io["df1"][:],
                          in_=ff1T[:].rearrange("p a b -> p (a b)"))

    psQ.release()

    # ======================================================================
    # Phase D: FFN2 over all 4 token-quarters, single w2 stream
    # ======================================================================
    with tc.tile_pool(name="ffn2p", bufs=1) as f2p, \
         tc.tile_pool(name="ps2", bufs=1, space="PSUM") as ps2p, \
         nc.named_scope("ffn2"):
        pso = [ps2p.tile([128, C], f32, tag="pso", bufs=4, name=f"pso_{tq}")
               for tq in range(4)]
        for mc in range(32):
            w2t = f2p.tile([128, C], bf16, tag="w2t", bufs=6, name=f"w2t_{mc}")
            nc.gpsimd.dma_start(out=w2t[:], in_=w2[mc * 128:(mc + 1) * 128, :])
            for tq in range(4):
                for eh in range(2):
                    nc.tensor.matmul(pso[tq][:, eh * 512:(eh + 1) * 512],
                                     ff1T[:, mc, tq * 128:(tq + 1) * 128],
                                     w2t[:, eh * 512:(eh + 1) * 512],
                                     start=(mc == 0), stop=False)
        for tq in range(4):
            for eh in range(2):
                nc.tensor.matmul(pso[tq][:, eh * 512:(eh + 1) * 512],
                                 ones_b, b2_row[0:1, eh * 512:(eh + 1) * 512],
                                 start=False, stop=True)
            ot = f2p.tile([128, C], f32, tag="ot", bufs=2, name=f"ot_{tq}")
            nc.vector.tensor_add(ot[:], pso[tq][:], x2[:, tq, :])
            nc.gpsimd.dma_start(out=out[tq * 128:(tq + 1) * 128, :], in_=ot[:])
    persD.release()
    persA.release()
    consts.release()
    dram.release()


# --------------------------------------------------------------------------
# host driver
# --------------------------------------------------------------------------
def _make_in_maps(inputs):
    bf = ml_dtypes.bfloat16
    x = np.ascontiguousarray(np.asarray(inputs["x"], np.float32))
    wq = np.asarray(inputs["wq"], np.float32)
    wk = np.asarray(inputs["wk"], np.float32)
    wv = np.asarray(inputs["wv"], np.float32)
    w_proj = np.asarray(inputs["w_proj"], np.float32)
    b_proj = np.asarray(inputs["b_proj"], np.float32)
    w1 = np.asarray(inputs["w1"], np.float32)
    b1 = np.asarray(inputs["b1"], np.float32)
    w2 = np.asarray(inputs["w2"], np.float32)
    b2 = np.asarray(inputs["b2"], np.float32)
    g1 = np.asarray(inputs["g1"], np.float32)
    be1 = np.asarray(inputs["be1"], np.float32)
    g2 = np.asarray(inputs["g2"], np.float32)
    be2 = np.asarray(inputs["be2"], np.float32)

    xf = x.reshape(BT, C)
    i_mask = np.zeros((4, 128, 512), np.float32)
    s_idx = np.arange(128)[:, None]
    t_idx = np.arange(512)[None, :]
    for i in range(4):
        i_mask[i] = (s_idx + 128 * i <= t_idx).astype(np.float32)

    w1f = g2[:, None] * w1                       # fold LN2 gamma
    b1f = b1 + be2 @ w1                          # fold LN2 beta

    common = dict(
        x_full=xf,
        x_bf=xf.astype(bf),
        masks=i_mask.astype(bf),
        cb=np.ascontiguousarray(b1f.reshape(32, 128).T),
        wproj=np.ascontiguousarray(w_proj).astype(bf),
        w1=np.ascontiguousarray(w1f).astype(bf),
        w2=np.ascontiguousarray(w2).astype(bf),
    )
    in_maps = []
    for c in range(NCORES):
        wqp = np.concatenate([wq[2 * c], wq[2 * c + 1]], axis=1)  # [C,128]
        wkp = np.concatenate([wk[2 * c], wk[2 * c + 1]], axis=1)
        wvp = np.concatenate([wv[2 * c], wv[2 * c + 1]], axis=1)
        wqkv = np.concatenate([g1[:, None] * wqp, g1[:, None] * wkp,
                               g1[:, None] * wvp], axis=1)        # [C,384]
        rows = np.concatenate([np.ones(512, np.float32), b_proj, b2,
                               be1 @ wqp, be1 @ wkp, be1 @ wvp])[None, :]
        m = dict(common)
        m["wqkv"] = np.ascontiguousarray(wqkv).astype(bf)
        m["rows"] = np.ascontiguousarray(rows).astype(bf)
        m["x_own"] = np.ascontiguousarray(
            np.concatenate([xf[TOK * c:TOK * (c + 1)],
                            xf[T + TOK * c:T + TOK * (c + 1)]], axis=0))
        in_maps.append(m)
    return in_maps


LAST_RESULTS = None


def kernel(trace=False, **inputs):
    global LAST_RESULTS
    from concourse import bass_utils

    if "nc" not in _CACHE:
        _CACHE["nc"] = _build_program()
    nc = _CACHE["nc"]
    in_maps = _make_in_maps(inputs)
    res = bass_utils.run_bass_kernel_spmd(
        nc, in_maps, core_ids=list(range(NCORES)), trace=trace)
    LAST_RESULTS = res
    out = np.zeros((B, T, C), np.float32)
    for c in range(NCORES):
        r = res.results[c]["out"]
        out[0, TOK * c:TOK * (c + 1), :] = r[0:TOK]
        out[1, TOK * c:TOK * (c + 1), :] = r[TOK:2 * TOK]
    return out


# revision 48
# speedup vs baseline: 1.3139x; 1.1735x over previous
"""Trainium2 Bass kernel for a dense pre-LN transformer block.

Problem: B=2, T=2048, C=1024, H=16 heads (d=64), FFN 4x, causal attention.

Parallelization over 8 NeuronCores (single SPMD program, one launch):
  - Attention phase: head-tensor-parallel. Core c computes heads {2c, 2c+1}
    for BOTH batches: LN1 (replicated), Q/K/V projections, causal-block
    attention with unnormalized softmax (denominator via an appended
    ones-column in V), normalization.
  - One 8-core AllToAll redistributes attn^T from head-split to
    (batch, token)-split: shard j carries the core's 2 head-rows for
    (batch j//4, token-quarter j%4).
  - Post-A2A phase: core c owns (batch c//4, tokens [c%4*512, ...+512)):
    output projection + residual, LN2, FFN, residual; returns its
    512x1024 slice of the output.

Perf notes on top of the original structure:
  - LayerNorm rsqrt computed on DVE (quake seed + 2 Newton steps): the
    scalar engine then only ever runs Exp/Relu, which share one
    activation-table set -> no ACT_TABLE_LOAD thrash.
  - LN statistics and normalization read a host-provided bf16 copy of x
    (2x DVE rate, half the HBM traffic); residuals still use f32 x.
  - h/h2 transposes are regular matmuls against a bf16 identity moving
    operand (~2.5x cheaper than transpose-mode).
  - Receiver-side softmax normalization uses reciprocal_approx_fast on an
    f32 copy instead of the multi-pass DVE reciprocal.
  - Constant loads are issued on the scalar queue so the x tiles own the
    sync DMA queue from t=0; FFN weight streams ride the GpSimd SWDGE
    queue.
"""

import numpy as np
import ml_dtypes

B, T, C = 2, 2048, 1024
H, D = 16, 64
FF = 4 * C
EPS = 1e-5
NCORES = 8
TSL = 512  # tokens owned per core in the post-A2A phase
BT = B * T

_CACHE = {}


# --------------------------------------------------------------------------
# device program
# --------------------------------------------------------------------------
def _build_program():
    import concourse.bass as bass
    import concourse.mybir as mybir
    import concourse.tile as tile
    from concourse import bacc

    dt = mybir.dt
    f32, f32r, bf16 = dt.float32, dt.float32r, dt.bfloat16

    nc = bacc.Bacc("TRN2", target_bir_lowering=False, debug=False,
                   num_devices=NCORES)

    # ---- I/O ----
    x_full = nc.dram_tensor("x_full", [BT, C], bf16, kind="ExternalInput")
    x_own = nc.dram_tensor("x_own", [TSL, C], f32, kind="ExternalInput")
    wq2 = nc.dram_tensor("wq2", [C, 128], bf16, kind="ExternalInput")
    wk2 = nc.dram_tensor("wk2", [C, 128], bf16, kind="ExternalInput")
    wv_aug = nc.dram_tensor("wv_aug", [C, 130], bf16, kind="ExternalInput")
    onespat = nc.dram_tensor("onespat", [1, 130], bf16, kind="ExternalInput")
    ones_b = nc.dram_tensor("ones_b", [1, 128], bf16, kind="ExternalInput")
    masks = nc.dram_tensor("masks", [4, 128, 512], bf16, kind="ExternalInput")
    wproj = nc.dram_tensor("wproj", [C, C], bf16, kind="ExternalInput")
    bproj = nc.dram_tensor("bproj", [1, C], bf16, kind="ExternalInput")
    w1 = nc.dram_tensor("w1", [C, FF], bf16, kind="ExternalInput")
    w2 = nc.dram_tensor("w2", [FF, C], bf16, kind="ExternalInput")
    b1t = nc.dram_tensor("b1t", [128, FF // 128], f32, kind="ExternalInput")
    b2row = nc.dram_tensor("b2row", [1, C], bf16, kind="ExternalInput")
    cbq = nc.dram_tensor("cbq", [128, 2], f32, kind="ExternalInput")
    identb = nc.dram_tensor("identb", [128, 128], bf16, kind="ExternalInput")
    out = nc.dram_tensor("out", [TSL, C], f32, kind="ExternalOutput")

    with tile.TileContext(nc, num_cores=NCORES) as tc:
        _body(nc, tc, tile, mybir, bass, locals())
    nc.compile()
    return nc


def _rsqrt_dve(nc, mybir, pool, var_ap, n, name):
    """rsqrt(var + EPS) on DVE: quake bit-trick seed + 2 Newton steps.

    var_ap: [128, n] f32 (may be strided). Returns a [128, n] f32 tile.
    Avoids the scalar engine so the activation table never leaves the
    exp set."""
    dt = mybir.dt
    f32, i32, u32 = dt.float32, dt.int32, dt.uint32
    OP = mybir.AluOpType
    vv = pool.tile([128, n], f32, tag="vv", bufs=2, name=f"vv_{name}")
    nc.vector.tensor_scalar_add(out=vv[:], in0=var_ap, scalar1=EPS)
    y = pool.tile([128, n], f32, tag="yy", bufs=2, name=f"yy_{name}")
    nc.vector.tensor_scalar(out=y[:].bitcast(u32), in0=vv[:].bitcast(u32),
                            scalar1=1, scalar2=None,
                            op0=OP.logical_shift_right)
    nc.vector.tensor_scalar(out=y[:].bitcast(i32), in0=y[:].bitcast(i32),
                            scalar1=0x5F3759DF, scalar2=-1,
                            op0=OP.subtract, op1=OP.mult)
    t = pool.tile([128, n], f32, tag="tt", bufs=2, name=f"tt_{name}")
    for _ in range(2):
        nc.vector.tensor_mul(t[:], y[:], y[:])
        nc.vector.scalar_tensor_tensor(out=t[:], in0=t[:], scalar=-0.5,
                                       in1=vv[:], op0=OP.mult, op1=OP.mult)
        nc.vector.scalar_tensor_tensor(out=y[:], in0=t[:], scalar=1.5,
                                       in1=y[:], op0=OP.add, op1=OP.mult)
    return y


def _body(nc, tc, tile, mybir, bass, io):
    dt = mybir.dt
    f32, f32r, bf16 = dt.float32, dt.float32r, dt.bfloat16
    AF = mybir.ActivationFunctionType
    OP = mybir.AluOpType

    x_full, x_own = io["x_full"], io["x_own"]
    wq2, wk2, wv_aug = io["wq2"], io["wk2"], io["wv_aug"]
    onespat, ones_b = io["onespat"], io["ones_b"]
    masks, wproj, bproj = io["masks"], io["wproj"], io["bproj"]
    w1, w2, b1t = io["w1"], io["w2"], io["b1t"]
    b2row = io["b2row"]
    cbq = io["cbq"]
    identb, out = io["identb"], io["out"]

    # ---- persistent pools ----
    consts = tc.alloc_tile_pool(name="consts", bufs=1)
    persA = tc.alloc_tile_pool(name="persA", bufs=1)  # attention lifetime
    dram = tc.alloc_tile_pool(name="dram", bufs=1, space="DRAM")

    # constants on the scalar queue: x tiles own the sync queue from t=0
    idb_sb = consts.tile([128, 128], bf16, name="idb_sb")
    nc.scalar.dma_start(out=idb_sb[:], in_=identb[:])
    wq_sb = consts.tile([128, 8, 128], bf16, name="wq_sb")
    nc.scalar.dma_start(out=wq_sb[:], in_=wq2[:].rearrange("(cc p) d -> p cc d", p=128))
    wk_sb = consts.tile([128, 8, 128], bf16, name="wk_sb")
    nc.scalar.dma_start(out=wk_sb[:], in_=wk2[:].rearrange("(cc p) d -> p cc d", p=128))
    wv_sb = consts.tile([128, 8, 130], bf16, name="wv_sb")
    nc.scalar.dma_start(out=wv_sb[:], in_=wv_aug[:].rearrange("(cc p) d -> p cc d", p=128))
    onespat_sb = consts.tile([1, 130], bf16, name="onespat_sb")
    nc.scalar.dma_start(out=onespat_sb[:], in_=onespat[:])
    ones_b_sb = consts.tile([1, 128], bf16, name="ones_b_sb")
    nc.scalar.dma_start(out=ones_b_sb[:], in_=ones_b[:])
    cbq_sb = consts.tile([128, 2], f32, name="cbq_sb")
    nc.scalar.dma_start(out=cbq_sb[:], in_=cbq[:])
    b1_sb = consts.tile([128, FF // 128], f32, name="b1_sb")
    nc.scalar.dma_start(out=b1_sb[:], in_=b1t[:])
    b2r_sb = consts.tile([1, C], bf16, name="b2r_sb")
    nc.scalar.dma_start(out=b2r_sb[:], in_=b2row[:])
    bproj_sb = consts.tile([1, C], bf16, name="bproj_sb")
    nc.scalar.dma_start(out=bproj_sb[:], in_=bproj[:])
    mask_sb = consts.tile([128, 4, 512], bf16, name="mask_sb")
    nc.scalar.dma_start(out=mask_sb[:], in_=masks[:].rearrange("i p t -> p i t"))

    # attention-persistent tensors, per batch (bf16 pipeline)
    qT = [persA.tile([128, T], bf16, name=f"qTb{b}") for b in range(2)]
    kT = [persA.tile([128, T], bf16, name=f"kTb{b}") for b in range(2)]
    vaug = [persA.tile([128, 16 * 130], bf16, name=f"vaugb{b}") for b in range(2)]
    aT_h = [[persA.tile([64, T], bf16, name=f"aTb{b}h{h}") for h in range(2)]
            for b in range(2)]

    a2a_in = [dram.tile([8, 65, 512], bf16, name=f"a2a_in{hh}")
              for hh in range(2)]
    a2a_out = [dram.tile([8, 65, 512], bf16, name=f"a2a_out{hh}")
               for hh in range(2)]

    # ======================================================================
    # Phases A+B fused scope: LN1+QKV (per batch) then causal attention.
    # Per-batch tiles let batch-1 projections overlap batch-0 attention.
    # ======================================================================
    with tc.tile_pool(name="lnAB", bufs=1) as lnA, \
         tc.tile_pool(name="psAB", bufs=1, space="PSUM") as psA:
        dens = [None, None]
        for b in range(2):
            with nc.named_scope(f"qkv_b{b}"):
                for tch in range(4):  # t-chunks of 512 within this batch
                    hsubs = []
                    mvt = lnA.tile([128, 2, 4], f32, tag="mvt", bufs=2,
                                   name=f"mvt_{b}_{tch}")
                    xsubs = []
                    for sub in range(4):
                        row0 = b * T + tch * 512 + sub * 128
                        xt = lnA.tile([128, C], bf16, tag="xt", bufs=6,
                                      name=f"xt_{b}_{tch}_{sub}")
                        nc.sync.dma_start(out=xt[:], in_=x_full[row0:row0 + 128, :])
                        xsubs.append(xt)
                        st = lnA.tile([128, 2, 6], f32, tag="st", bufs=2,
                                      name=f"st_{b}_{tch}_{sub}")
                        nc.vector.bn_stats(out=st[:, 0, :], in_=xt[:, 0:512])
                        nc.vector.bn_stats(out=st[:, 1, :], in_=xt[:, 512:1024])
                        nc.vector.bn_aggr(out=mvt[:, :, sub], in_=st[:])
                    rsq = _rsqrt_dve(nc, mybir, lnA, mvt[:, 1, :], 4,
                                     f"{b}_{tch}")
                    for sub in range(4):
                        h = lnA.tile([128, C], bf16, tag="h", bufs=5,
                                     name=f"h_{b}_{tch}_{sub}")
                        nc.vector.tensor_scalar(out=h[:], in0=xsubs[sub][:],
                                                scalar1=mvt[:, 0, sub:sub + 1],
                                                scalar2=rsq[:, sub:sub + 1],
                                                op0=OP.subtract, op1=OP.mult)
                        hsubs.append(h)
                    # transpose h (bf16) -> hTb [c, t] via regular matmuls
                    # against the identity moving operand; LN1 affine in copy
                    hTb = lnA.tile([128, 8, 512], bf16, tag="hTb", bufs=2,
                                   name=f"hTb_{b}_{tch}")
                    for cc in range(8):
                        pth = psA.tile([128, 512], f32, tag="pqv", bufs=2,
                                       name=f"pth_{b}_{tch}_{cc}")
                        for sub in range(4):
                            nc.tensor.matmul(
                                pth[:, sub * 128:(sub + 1) * 128],
                                hsubs[sub][:, cc * 128:(cc + 1) * 128],
                                idb_sb[:], start=True, stop=True)
                        nc.scalar.copy(out=hTb[:, cc, :], in_=pth[:])
                    # q^T, k^T (bf16 matmul -> bf16 store)
                    col = tch * 512
                    for wi, (w_sb, dst) in enumerate(((wq_sb, qT[b]),
                                                      (wk_sb, kT[b]))):
                        pqk = psA.tile([128, 512], f32, tag="pqv", bufs=2,
                                       name=f"pqk_{b}_{tch}_{wi}")
                        for cc in range(8):
                            nc.tensor.matmul(pqk[:], w_sb[:, cc, :],
                                             hTb[:, cc, :],
                                             start=(cc == 0), stop=(cc == 7))
                        nc.vector.tensor_scalar_add(
                            out=dst[:, col:col + 512], in0=pqk[:],
                            scalar1=cbq_sb[:, wi:wi + 1])
                    # v (+ ones column), bf16
                    for sub in range(4):
                        sb = tch * 4 + sub
                        pv = psA.tile([128, 512], f32, tag="pqv", bufs=2,
                                      name=f"pv_{b}_{sb}")
                        for cc in range(8):
                            nc.tensor.matmul(
                                pv[:, 0:130], hTb[:, cc, sub * 128:(sub + 1) * 128],
                                wv_sb[:, cc, :], start=(cc == 0), stop=False)
                        nc.tensor.matmul(pv[:, 0:130], ones_b_sb[:], onespat_sb[:],
                                         start=False, stop=True)
                        nc.vector.tensor_copy(out=vaug[b][:, sb * 130:(sb + 1) * 130],
                                              in_=pv[:, 0:130])
        # ---- attention h-major: head-0 A2A hides under head-1 attention ----
        for b in range(2):
            dens[b] = lnA.tile([65, 2, T], bf16, tag="den", bufs=2,
                               name=f"den_{b}")  # row 64: softmax denominators
        for h in range(2):
            hp = 64 * h
            for b in range(2):
                den = dens[b]
                with nc.named_scope(f"attn_b{b}h{h}"):
                    for half in range(2):  # query chunks (2*half, 2*half+1)
                        qc0 = half * 1024
                        pat = [psA.tile([65, 512], f32, tag="pat", bufs=2,
                                        name=f"pat_{b}_{h}_{half}_{i}")
                               for i in range(2)]
                        nsb = 8 * half + 8
                        for sb in range(nsb):
                            # active query chunks of this half (causal)
                            act0 = 0 if sb < 8 * half + 4 else 1
                            dtc = sb // 4 - 2 * half  # diag chunk idx in half
                            ps = psA.tile([128, 1024], f32, tag="ps", bufs=2,
                                          name=f"ps_{b}_{h}_{half}_{sb}")
                            for i in range(act0, 2):
                                nc.tensor.matmul(
                                    ps[:, i * 512:(i + 1) * 512],
                                    kT[b][hp:hp + 64, sb * 128:sb * 128 + 128],
                                    qT[b][hp:hp + 64,
                                          qc0 + i * 512:qc0 + (i + 1) * 512],
                                    start=True, stop=True)
                            pt = lnA.tile([128, 1024], bf16, tag="pt", bufs=4,
                                          name=f"pt_{b}_{h}_{half}_{sb}")
                            nc.scalar.activation(out=pt[:, act0 * 512:1024],
                                                 in_=ps[:, act0 * 512:1024],
                                                 func=AF.Exp, scale=0.125)
                            if dtc >= act0:
                                nc.vector.tensor_mul(
                                    pt[:, dtc * 512:(dtc + 1) * 512],
                                    pt[:, dtc * 512:(dtc + 1) * 512],
                                    mask_sb[:, sb % 4, :])
                            vs = sb * 130 + 65 * h
                            for i in range(act0, 2):
                                last = 8 * half + 3 if i == 0 else nsb - 1
                                nc.tensor.matmul(
                                    pat[i][:], vaug[b][:, vs:vs + 65],
                                    pt[:, i * 512:(i + 1) * 512],
                                    start=(sb == 0), stop=(sb == last))
                        for i in range(2):
                            tc4 = 2 * half + i
                            qcol = tc4 * 512
                            nc.vector.tensor_copy(
                                out=aT_h[b][h][:, qcol:qcol + 512],
                                in_=pat[i][0:64, :])
                            nc.vector.tensor_copy(
                                out=den[64:65, h, qcol:qcol + 512],
                                in_=pat[i][64:65, :])
            # shard DMAs + collective for this head (first one overlaps the
            # second head's attention)
            for j in range(8):
                bj, tq = j // 4, j % 4
                scol = tq * 512
                nc.sync.dma_start(out=a2a_in[h][j, 0:64, :],
                                  in_=aT_h[bj][h][:, scol:scol + 512])
                nc.sync.dma_start(out=a2a_in[h][j, 64:65, :],
                                  in_=dens[bj][64:65, h, scol:scol + 512])
            nc.gpsimd.collective_compute(
                "AllToAll", mybir.AluOpType.bypass,
                replica_groups=[list(range(NCORES))],
                ins=[a2a_in[h][:].opt()], outs=[a2a_out[h][:].opt()])
    persA.release()

    # ======================================================================
    # Phases D+E fused scope: projection + residual + LN2 + FFN + output
    # ======================================================================
    persD = tc.alloc_tile_pool(name="persD", bufs=1)
    x2 = persD.tile([128, 4, C], f32, name="x2")
    h2T = persD.tile([128, 8, 512], bf16, name="h2T")
    ff1T = persD.tile([128, 32, 512], bf16, name="ff1T")
    w1r = w1[:].rearrange("(cc p) m -> p cc m", p=128)
    with tc.tile_pool(name="prDE", bufs=1) as prD:
        aT_own = prD.tile([128, 8, 512], bf16, tag="aT_own", name="aT_own")
        for r in range(8):
            nc.gpsimd.dma_start(out=aT_own[0:64, r, :],
                                in_=a2a_out[0][r, 0:64, :])
            nc.gpsimd.dma_start(out=aT_own[64:128, r, :],
                                in_=a2a_out[1][r, 0:64, :])
        # receiver-side softmax normalization: r-th block rows scale by
        # 1/den of heads {2r, 2r+1} (denoms rode each A2A as row 64)
        for r in range(8):
            rb = prD.tile([128, 512], bf16, tag="rb", bufs=3, name=f"rb_{r}")
            nc.gpsimd.dma_start(out=rb[0:64, :],
                                in_=a2a_out[0][r, 64:65, :].to_broadcast([64, 512]))
            nc.gpsimd.dma_start(out=rb[64:128, :],
                                in_=a2a_out[1][r, 64:65, :].to_broadcast([64, 512]))
            rc = prD.tile([128, 512], f32, tag="rc", bufs=3, name=f"rc_{r}")
            nc.vector.tensor_copy(out=rc[:], in_=rb[:])
            rf = prD.tile([128, 512], f32, tag="rf", bufs=3, name=f"rf_{r}")
            nc.vector.reciprocal_approx_fast(out=rf[:], in_=rc[:])
            nc.vector.tensor_mul(aT_own[:, r, :], aT_own[:, r, :], rf[:])
        wp_sb = prD.tile([128, 8, C], bf16, tag="wp_sb", name="wp_sb")
        nc.scalar.dma_start(out=wp_sb[:],
                            in_=wproj[:].rearrange("(dc p) e -> p dc e", p=128))
        xo = prD.tile([128, 4, C], f32, tag="xo", name="xo")
        nc.scalar.dma_start(out=xo[:],
                            in_=x_own[:].rearrange("(tq p) e -> p tq e", p=128))
        with tc.tile_pool(name="psD", bufs=1, space="PSUM") as psD, \
             nc.named_scope("proj_ln2"):
            h2subs = []
            mv2t = prD.tile([128, 2, 4], f32, tag="mv2t", name="mv2t")
            for tq in range(4):
                for eh in range(2):
                    pp = psD.tile([128, 512], f32, tag="pp", bufs=2,
                                  name=f"pp_{tq}_{eh}")
                    for dc in range(8):
                        nc.tensor.matmul(
                            pp[:], aT_own[:, dc, tq * 128:(tq + 1) * 128],
                            wp_sb[:, dc, eh * 512:eh * 512 + 512],
                            start=(dc == 0), stop=False)
                    nc.tensor.matmul(pp[:], ones_b_sb[:],
                                     bproj_sb[0:1, eh * 512:eh * 512 + 512],
                                     start=False, stop=True)
                    nc.vector.tensor_add(x2[:, tq, eh * 512:eh * 512 + 512],
                                         pp[:], xo[:, tq, eh * 512:eh * 512 + 512])
                st2 = prD.tile([128, 2, 6], f32, tag="st2", bufs=2,
                               name=f"st2_{tq}")
                nc.vector.bn_stats(out=st2[:, 0, :], in_=x2[:, tq, 0:512])
                nc.vector.bn_stats(out=st2[:, 1, :], in_=x2[:, tq, 512:1024])
                nc.vector.bn_aggr(out=mv2t[:, :, tq], in_=st2[:])
            rsq2 = _rsqrt_dve(nc, mybir, prD, mv2t[:, 1, :], 4, "ln2")
            for tq in range(4):
                h2 = prD.tile([128, C], bf16, tag="h2", bufs=5, name=f"h2_{tq}")
                nc.vector.tensor_scalar(out=h2[:], in0=x2[:, tq, :],
                                        scalar1=mv2t[:, 0, tq:tq + 1],
                                        scalar2=rsq2[:, tq:tq + 1],
                                        op0=OP.subtract, op1=OP.mult)
                h2subs.append(h2)
            for cc in range(8):
                pt2 = psD.tile([128, 512], f32, tag="pp", bufs=2,
                               name=f"pt2_{cc}")
                for tq in range(4):
                    nc.tensor.matmul(pt2[:, tq * 128:(tq + 1) * 128],
                                     h2subs[tq][:, cc * 128:(cc + 1) * 128],
                                     idb_sb[:], start=True, stop=True)
                nc.scalar.copy(out=h2T[:, cc, :], in_=pt2[:])
        with tc.tile_pool(name="ps1", bufs=1, space="PSUM") as ps1, \
             nc.named_scope("ffn1"):
            for w in range(16):  # m-windows of 256
                w1w = prD.tile([128, 8, 256], bf16, tag="w1w", bufs=3,
                               name=f"w1w_{w}")
                nc.gpsimd.dma_start(out=w1w[:], in_=w1r[:, :, w * 256:(w + 1) * 256])
                for m2 in range(2):
                    m = w * 2 + m2  # m-chunk of 128
                    pf = ps1.tile([128, 512], f32, tag="pf", bufs=3,
                                  name=f"pf_{m}")
                    for cc in range(8):
                        nc.tensor.matmul(
                            pf[:], w1w[:, cc, m2 * 128:(m2 + 1) * 128],
                            h2T[:, cc, :], start=(cc == 0), stop=(cc == 7))
                    nc.scalar.activation(out=ff1T[:, m, :], in_=pf[:],
                                         func=AF.Relu, bias=b1_sb[:, m:m + 1])
        # FFN2 with ff1T stationary / w2 rows moving (1024-wide bf16):
        # output lands directly as ff[t, e]; no transposes needed.
        with tc.tile_pool(name="ps2", bufs=1, space="PSUM") as ps2p, \
             nc.named_scope("ffn2"):
            pso = [ps2p.tile([128, C], f32, tag="pso", bufs=4, name=f"pso_{tq}")
                   for tq in range(4)]
            for mc in range(32):
                w2t = prD.tile([128, C], bf16, tag="w2t", bufs=6,
                               name=f"w2t_{mc}")
                nc.gpsimd.dma_start(out=w2t[:], in_=w2[mc * 128:(mc + 1) * 128, :])
                for tq in range(4):
                    for eh in range(2):
                        nc.tensor.matmul(pso[tq][:, eh * 512:(eh + 1) * 512],
                                         ff1T[:, mc, tq * 128:(tq + 1) * 128],
                                         w2t[:, eh * 512:(eh + 1) * 512],
                                         start=(mc == 0), stop=False)
            for tq in range(4):
                for eh in range(2):
                    nc.tensor.matmul(pso[tq][:, eh * 512:(eh + 1) * 512],
                                     ones_b_sb[:], b2r_sb[0:1, eh * 512:(eh + 1) * 512],
                                     start=False, stop=True)
                ot = prD.tile([128, C], f32, tag="ot", bufs=2, name=f"ot_{tq}")
                nc.vector.tensor_add(ot[:], pso[tq][:], x2[:, tq, :])
                nc.sync.dma_start(out=out[tq * 128:(tq + 1) * 128, :], in_=ot[:])
    persD.release()
    consts.release()
    dram.release()


# --------------------------------------------------------------------------
# host driver
# --------------------------------------------------------------------------
def _make_in_maps(inputs):
    x = np.ascontiguousarray(np.asarray(inputs["x"], np.float32))
    wq = np.asarray(inputs["wq"], np.float32)
    wk = np.asarray(inputs["wk"], np.float32)
    wv = np.asarray(inputs["wv"], np.float32)
    w_proj = np.ascontiguousarray(np.asarray(inputs["w_proj"], np.float32))
    b_proj = np.asarray(inputs["b_proj"], np.float32)
    w1 = np.ascontiguousarray(np.asarray(inputs["w1"], np.float32))
    b1 = np.asarray(inputs["b1"], np.float32)
    w2 = np.ascontiguousarray(np.asarray(inputs["w2"], np.float32))
    b2 = np.asarray(inputs["b2"], np.float32)
    g1 = np.asarray(inputs["g1"], np.float32)
    be1 = np.asarray(inputs["be1"], np.float32)
    g2 = np.asarray(inputs["g2"], np.float32)
    be2 = np.asarray(inputs["be2"], np.float32)

    xf = x.reshape(BT, C)
    b1 = b1 + be2 @ w1          # fold LN2 beta (uses original w1)
    w1 = np.ascontiguousarray(g2[:, None] * w1)   # fold LN2 gamma

    i_mask = np.zeros((4, 128, 512), np.float32)
    s_idx = np.arange(128)[:, None]
    t_idx = np.arange(512)[None, :]
    for i in range(4):
        i_mask[i] = (s_idx + 128 * i <= t_idx).astype(np.float32)
    onespat = np.zeros((1, 130), np.float32)
    onespat[0, 64] = 1.0
    onespat[0, 129] = 1.0

    common = dict(
        x_full=xf.astype(ml_dtypes.bfloat16),
        masks=i_mask.astype(ml_dtypes.bfloat16),
        onespat=onespat.astype(ml_dtypes.bfloat16),
        ones_b=np.ones((1, 128), ml_dtypes.bfloat16),
        wproj=w_proj.astype(ml_dtypes.bfloat16),
        bproj=np.ascontiguousarray(b_proj[None, :]).astype(ml_dtypes.bfloat16),
        w1=w1.astype(ml_dtypes.bfloat16), w2=w2.astype(ml_dtypes.bfloat16),
        b1t=np.ascontiguousarray(b1.reshape(FF // 128, 128).T),
        b2row=np.ascontiguousarray(b2[None, :]).astype(ml_dtypes.bfloat16),
        identb=np.eye(128).astype(ml_dtypes.bfloat16),
    )
    in_maps = []
    for c in range(NCORES):
        b, hg = c // 4, c % 4
        wva = np.zeros((C, 130), np.float32)
        wva[:, 0:64] = wv[2 * c]
        wva[:, 65:129] = wv[2 * c + 1]
        wq2 = np.concatenate([wq[2 * c], wq[2 * c + 1]], axis=1)
        wk2 = np.concatenate([wk[2 * c], wk[2 * c + 1]], axis=1)
        m = dict(common)
        m["x_own"] = np.ascontiguousarray(
            xf[b * T + hg * TSL: b * T + (hg + 1) * TSL])
        m["wq2"] = np.ascontiguousarray(g1[:, None] * wq2).astype(
            ml_dtypes.bfloat16)
        m["wk2"] = np.ascontiguousarray(g1[:, None] * wk2).astype(
            ml_dtypes.bfloat16)
        m["wv_aug"] = (g1[:, None] * wva).astype(ml_dtypes.bfloat16)
        m["onespat"] = np.ascontiguousarray(
            (be1 @ wva + onespat[0])[None, :]).astype(ml_dtypes.bfloat16)
        m["cbq"] = np.ascontiguousarray(
            np.stack([be1 @ wq2, be1 @ wk2], axis=1))
        in_maps.append(m)
    return in_maps


LAST_RESULTS = None


def kernel(trace=False, **inputs):
    global LAST_RESULTS
    from concourse import bass_utils

    if "nc" not in _CACHE:
        _CACHE["nc"] = _build_program()
    nc = _CACHE["nc"]
    in_maps = _make_in_maps(inputs)
    res = bass_utils.run_bass_kernel_spmd(
        nc, in_maps, core_ids=list(range(NCORES)), trace=trace)
    LAST_RESULTS = res
    out = np.zeros((B, T, C), np.float32)
    for c in range(NCORES):
        b, hg = c // 4, c % 4
        out[b, hg * TSL:(hg + 1) * TSL, :] = res.results[c]["out"]
    return out


# revision 50
# speedup vs baseline: 1.3421x; 1.0214x over previous
"""Trainium2 Bass kernel for a dense pre-LN transformer block.

Problem: B=2, T=2048, C=1024, H=16 heads (d=64), FFN 4x, causal attention.

Parallelization over 8 NeuronCores (single SPMD program, one launch):
  - Attention phase: head-tensor-parallel. Core c computes heads {2c, 2c+1}
    for BOTH batches: LN1 (replicated), Q/K/V projections, causal-block
    attention with unnormalized softmax (denominator via an appended
    ones-column in V), normalization.
  - One 8-core AllToAll redistributes attn^T from head-split to
    (batch, token)-split: shard j carries the core's 2 head-rows for
    (batch j//4, token-quarter j%4).
  - Post-A2A phase: core c owns (batch c//4, tokens [c%4*512, ...+512)):
    output projection + residual, LN2, FFN, residual; returns its
    512x1024 slice of the output.

Perf notes on top of the original structure:
  - LayerNorm rsqrt computed on DVE (quake seed + 2 Newton steps): the
    scalar engine then only ever runs Exp/Relu, which share one
    activation-table set -> no ACT_TABLE_LOAD thrash.
  - LN statistics and normalization read a host-provided bf16 copy of x
    (2x DVE rate, half the HBM traffic); residuals still use f32 x.
  - h/h2 transposes are regular matmuls against a bf16 identity moving
    operand (~2.5x cheaper than transpose-mode).
  - Receiver-side softmax normalization uses reciprocal_approx_fast on an
    f32 copy instead of the multi-pass DVE reciprocal.
  - Constant loads are issued on the scalar queue so the x tiles own the
    sync DMA queue from t=0; FFN weight streams ride the GpSimd SWDGE
    queue.
"""

import numpy as np
import ml_dtypes

B, T, C = 2, 2048, 1024
H, D = 16, 64
FF = 4 * C
EPS = 1e-5
NCORES = 8
TSL = 512  # tokens owned per core in the post-A2A phase
BT = B * T

_CACHE = {}


# --------------------------------------------------------------------------
# device program
# --------------------------------------------------------------------------
def _build_program():
    import concourse.bass as bass
    import concourse.mybir as mybir
    import concourse.tile as tile
    from concourse import bacc

    dt = mybir.dt
    f32, f32r, bf16 = dt.float32, dt.float32r, dt.bfloat16

    nc = bacc.Bacc("TRN2", target_bir_lowering=False, debug=False,
                   num_devices=NCORES)

    # ---- I/O ----
    x_full = nc.dram_tensor("x_full", [BT, C], bf16, kind="ExternalInput")
    x_own = nc.dram_tensor("x_own", [TSL, C], f32, kind="ExternalInput")
    wq2 = nc.dram_tensor("wq2", [C, 128], bf16, kind="ExternalInput")
    wk2 = nc.dram_tensor("wk2", [C, 128], bf16, kind="ExternalInput")
    wv_aug = nc.dram_tensor("wv_aug", [C, 130], bf16, kind="ExternalInput")
    onespat = nc.dram_tensor("onespat", [1, 130], bf16, kind="ExternalInput")
    ones_b = nc.dram_tensor("ones_b", [1, 128], bf16, kind="ExternalInput")
    masks = nc.dram_tensor("masks", [4, 128, 512], bf16, kind="ExternalInput")
    wproj = nc.dram_tensor("wproj", [C, C], bf16, kind="ExternalInput")
    bproj = nc.dram_tensor("bproj", [1, C], bf16, kind="ExternalInput")
    w1 = nc.dram_tensor("w1", [C, FF], bf16, kind="ExternalInput")
    w2 = nc.dram_tensor("w2", [FF, C], bf16, kind="ExternalInput")
    b1t = nc.dram_tensor("b1t", [128, FF // 128], f32, kind="ExternalInput")
    b2row = nc.dram_tensor("b2row", [1, C], bf16, kind="ExternalInput")
    cbq = nc.dram_tensor("cbq", [128, 2], f32, kind="ExternalInput")
    identb = nc.dram_tensor("identb", [128, 128], bf16, kind="ExternalInput")
    out = nc.dram_tensor("out", [TSL, C], f32, kind="ExternalOutput")

    with tile.TileContext(nc, num_cores=NCORES) as tc:
        _body(nc, tc, tile, mybir, bass, locals())
    nc.compile()
    return nc


def _rsqrt_dve(nc, mybir, pool, var_ap, n, name):
    """rsqrt(var + EPS) on DVE: quake bit-trick seed + 2 Newton steps.

    var_ap: [128, n] f32 (may be strided). Returns a [128, n] f32 tile.
    Avoids the scalar engine so the activation table never leaves the
    exp set."""
    dt = mybir.dt
    f32, i32, u32 = dt.float32, dt.int32, dt.uint32
    OP = mybir.AluOpType
    vv = pool.tile([128, n], f32, tag="vv", bufs=2, name=f"vv_{name}")
    nc.vector.tensor_scalar_add(out=vv[:], in0=var_ap, scalar1=EPS)
    y = pool.tile([128, n], f32, tag="yy", bufs=2, name=f"yy_{name}")
    nc.vector.tensor_scalar(out=y[:].bitcast(u32), in0=vv[:].bitcast(u32),
                            scalar1=1, scalar2=None,
                            op0=OP.logical_shift_right)
    nc.vector.tensor_scalar(out=y[:].bitcast(i32), in0=y[:].bitcast(i32),
                            scalar1=0x5F3759DF, scalar2=-1,
                            op0=OP.subtract, op1=OP.mult)
    t = pool.tile([128, n], f32, tag="tt", bufs=2, name=f"tt_{name}")
    for _ in range(2):
        nc.vector.tensor_mul(t[:], y[:], y[:])
        nc.vector.scalar_tensor_tensor(out=t[:], in0=t[:], scalar=-0.5,
                                       in1=vv[:], op0=OP.mult, op1=OP.mult)
        nc.vector.scalar_tensor_tensor(out=y[:], in0=t[:], scalar=1.5,
                                       in1=y[:], op0=OP.add, op1=OP.mult)
    return y


def _body(nc, tc, tile, mybir, bass, io):
    dt = mybir.dt
    f32, f32r, bf16 = dt.float32, dt.float32r, dt.bfloat16
    AF = mybir.ActivationFunctionType
    OP = mybir.AluOpType

    x_full, x_own = io["x_full"], io["x_own"]
    wq2, wk2, wv_aug = io["wq2"], io["wk2"], io["wv_aug"]
    onespat, ones_b = io["onespat"], io["ones_b"]
    masks, wproj, bproj = io["masks"], io["wproj"], io["bproj"]
    w1, w2, b1t = io["w1"], io["w2"], io["b1t"]
    b2row = io["b2row"]
    cbq = io["cbq"]
    identb, out = io["identb"], io["out"]

    # ---- persistent pools ----
    consts = tc.alloc_tile_pool(name="consts", bufs=1)
    persA = tc.alloc_tile_pool(name="persA", bufs=1)  # attention lifetime
    dram = tc.alloc_tile_pool(name="dram", bufs=1, space="DRAM")

    # constants on the scalar queue: x tiles own the sync queue from t=0
    idb_sb = consts.tile([128, 128], bf16, name="idb_sb")
    nc.scalar.dma_start(out=idb_sb[:], in_=identb[:])
    wq_sb = consts.tile([128, 8, 128], bf16, name="wq_sb")
    nc.scalar.dma_start(out=wq_sb[:], in_=wq2[:].rearrange("(cc p) d -> p cc d", p=128))
    wk_sb = consts.tile([128, 8, 128], bf16, name="wk_sb")
    nc.scalar.dma_start(out=wk_sb[:], in_=wk2[:].rearrange("(cc p) d -> p cc d", p=128))
    wv_sb = consts.tile([128, 8, 130], bf16, name="wv_sb")
    nc.scalar.dma_start(out=wv_sb[:], in_=wv_aug[:].rearrange("(cc p) d -> p cc d", p=128))
    onespat_sb = consts.tile([1, 130], bf16, name="onespat_sb")
    nc.scalar.dma_start(out=onespat_sb[:], in_=onespat[:])
    ones_b_sb = consts.tile([1, 128], bf16, name="ones_b_sb")
    nc.scalar.dma_start(out=ones_b_sb[:], in_=ones_b[:])
    cbq_sb = consts.tile([128, 2], f32, name="cbq_sb")
    nc.scalar.dma_start(out=cbq_sb[:], in_=cbq[:])
    b1_sb = consts.tile([128, FF // 128], f32, name="b1_sb")
    nc.scalar.dma_start(out=b1_sb[:], in_=b1t[:])
    b2r_sb = consts.tile([1, C], bf16, name="b2r_sb")
    nc.scalar.dma_start(out=b2r_sb[:], in_=b2row[:])
    bproj_sb = consts.tile([1, C], bf16, name="bproj_sb")
    nc.scalar.dma_start(out=bproj_sb[:], in_=bproj[:])
    mask_sb = consts.tile([128, 4, 512], bf16, name="mask_sb")
    nc.scalar.dma_start(out=mask_sb[:], in_=masks[:].rearrange("i p t -> p i t"))

    # attention-persistent tensors, per batch (bf16 pipeline)
    qT = [persA.tile([128, T], bf16, name=f"qTb{b}") for b in range(2)]
    kT = [persA.tile([128, T], bf16, name=f"kTb{b}") for b in range(2)]
    vaug = [persA.tile([128, 16 * 130], bf16, name=f"vaugb{b}") for b in range(2)]
    aT_h = [[persA.tile([64, T], bf16, name=f"aTb{b}h{h}") for h in range(2)]
            for b in range(2)]

    a2a_in = [dram.tile([8, 65, 512], bf16, name=f"a2a_in{hh}")
              for hh in range(2)]
    a2a_out = [dram.tile([8, 65, 512], bf16, name=f"a2a_out{hh}")
               for hh in range(2)]

    # ======================================================================
    # Phases A+B fused scope: LN1+QKV (per batch) then causal attention.
    # Per-batch tiles let batch-1 projections overlap batch-0 attention.
    # ======================================================================
    with tc.tile_pool(name="lnAB", bufs=1) as lnA, \
         tc.tile_pool(name="psAB", bufs=1, space="PSUM") as psA:
        dens = [None, None]
        for b in range(2):
            with nc.named_scope(f"qkv_b{b}"):
                for tch in range(4):  # t-chunks of 512 within this batch
                    hsubs = []
                    mvt = lnA.tile([128, 2, 4], f32, tag="mvt", bufs=2,
                                   name=f"mvt_{b}_{tch}")
                    xsubs = []
                    for sub in range(4):
                        row0 = b * T + tch * 512 + sub * 128
                        xt = lnA.tile([128, C], bf16, tag="xt", bufs=6,
                                      name=f"xt_{b}_{tch}_{sub}")
                        nc.sync.dma_start(out=xt[:], in_=x_full[row0:row0 + 128, :])
                        xsubs.append(xt)
                        st = lnA.tile([128, 2, 6], f32, tag="st", bufs=2,
                                      name=f"st_{b}_{tch}_{sub}")
                        nc.vector.bn_stats(out=st[:, 0, :], in_=xt[:, 0:512])
                        nc.vector.bn_stats(out=st[:, 1, :], in_=xt[:, 512:1024])
                        nc.vector.bn_aggr(out=mvt[:, :, sub], in_=st[:])
                    rsq = _rsqrt_dve(nc, mybir, lnA, mvt[:, 1, :], 4,
                                     f"{b}_{tch}")
                    for sub in range(4):
                        h = lnA.tile([128, C], bf16, tag="h", bufs=5,
                                     name=f"h_{b}_{tch}_{sub}")
                        nc.vector.tensor_scalar(out=h[:], in0=xsubs[sub][:],
                                                scalar1=mvt[:, 0, sub:sub + 1],
                                                scalar2=rsq[:, sub:sub + 1],
                                                op0=OP.subtract, op1=OP.mult)
                        hsubs.append(h)
                    # transpose h (bf16) -> hTb [c, t] via regular matmuls
                    # against the identity moving operand; LN1 affine in copy
                    hTb = lnA.tile([128, 8, 512], bf16, tag="hTb", bufs=2,
                                   name=f"hTb_{b}_{tch}")
                    for cc in range(8):
                        pth = psA.tile([128, 512], f32, tag="pqv", bufs=2,
                                       name=f"pth_{b}_{tch}_{cc}")
                        for sub in range(4):
                            nc.tensor.matmul(
                                pth[:, sub * 128:(sub + 1) * 128],
                                hsubs[sub][:, cc * 128:(cc + 1) * 128],
                                idb_sb[:], start=True, stop=True)
                        nc.scalar.copy(out=hTb[:, cc, :], in_=pth[:])
                    # q^T, k^T (bf16 matmul -> bf16 store)
                    col = tch * 512
                    for wi, (w_sb, dst) in enumerate(((wq_sb, qT[b]),
                                                      (wk_sb, kT[b]))):
                        pqk = psA.tile([128, 512], f32, tag="pqv", bufs=2,
                                       name=f"pqk_{b}_{tch}_{wi}")
                        for cc in range(8):
                            nc.tensor.matmul(pqk[:], w_sb[:, cc, :],
                                             hTb[:, cc, :],
                                             start=(cc == 0), stop=(cc == 7))
                        nc.vector.tensor_scalar_add(
                            out=dst[:, col:col + 512], in0=pqk[:],
                            scalar1=cbq_sb[:, wi:wi + 1])
                    # v (+ ones column), bf16
                    for sub in range(4):
                        sb = tch * 4 + sub
                        pv = psA.tile([128, 512], f32, tag="pqv", bufs=2,
                                      name=f"pv_{b}_{sb}")
                        for cc in range(8):
                            nc.tensor.matmul(
                                pv[:, 0:130], hTb[:, cc, sub * 128:(sub + 1) * 128],
                                wv_sb[:, cc, :], start=(cc == 0), stop=False)
                        nc.tensor.matmul(pv[:, 0:130], ones_b_sb[:], onespat_sb[:],
                                         start=False, stop=True)
                        nc.vector.tensor_copy(out=vaug[b][:, sb * 130:(sb + 1) * 130],
                                              in_=pv[:, 0:130])
        # ---- attention h-major: head-0 A2A hides under head-1 attention ----
        for b in range(2):
            dens[b] = lnA.tile([65, 2, T], bf16, tag="den", bufs=2,
                               name=f"den_{b}")  # row 64: softmax denominators
        for h in range(2):
            hp = 64 * h
            for b in range(2):
                den = dens[b]
                with nc.named_scope(f"attn_b{b}h{h}"):
                    for half in range(2):  # query chunks (2*half, 2*half+1)
                        qc0 = half * 1024
                        pat = [psA.tile([65, 512], f32, tag="pat", bufs=2,
                                        name=f"pat_{b}_{h}_{half}_{i}")
                               for i in range(2)]
                        nsb = 8 * half + 8
                        for sb in range(nsb):
                            # active query chunks of this half (causal)
                            act0 = 0 if sb < 8 * half + 4 else 1
                            dtc = sb // 4 - 2 * half  # diag chunk idx in half
                            ps = psA.tile([128, 1024], f32, tag="ps", bufs=2,
                                          name=f"ps_{b}_{h}_{half}_{sb}")
                            for i in range(act0, 2):
                                nc.tensor.matmul(
                                    ps[:, i * 512:(i + 1) * 512],
                                    kT[b][hp:hp + 64, sb * 128:sb * 128 + 128],
                                    qT[b][hp:hp + 64,
                                          qc0 + i * 512:qc0 + (i + 1) * 512],
                                    start=True, stop=True)
                            pt = lnA.tile([128, 1024], bf16, tag="pt", bufs=4,
                                          name=f"pt_{b}_{h}_{half}_{sb}")
                            nc.scalar.activation(out=pt[:, act0 * 512:1024],
                                                 in_=ps[:, act0 * 512:1024],
                                                 func=AF.Exp, scale=0.125)
                            if dtc >= act0:
                                nc.vector.tensor_mul(
                                    pt[:, dtc * 512:(dtc + 1) * 512],
                                    pt[:, dtc * 512:(dtc + 1) * 512],
                                    mask_sb[:, sb % 4, :])
                            vs = sb * 130 + 65 * h
                            for i in range(act0, 2):
                                last = 8 * half + 3 if i == 0 else nsb - 1
                                nc.tensor.matmul(
                                    pat[i][:], vaug[b][:, vs:vs + 65],
                                    pt[:, i * 512:(i + 1) * 512],
                                    start=(sb == 0), stop=(sb == last))
                        for i in range(2):
                            tc4 = 2 * half + i
                            qcol = tc4 * 512
                            nc.vector.tensor_copy(
                                out=aT_h[b][h][:, qcol:qcol + 512],
                                in_=pat[i][0:64, :])
                            nc.vector.tensor_copy(
                                out=den[64:65, h, qcol:qcol + 512],
                                in_=pat[i][64:65, :])
            # shard DMAs + collective for this head (first one overlaps the
            # second head's attention)
            for j in range(8):
                bj, tq = j // 4, j % 4
                scol = tq * 512
                nc.sync.dma_start(out=a2a_in[h][j, 0:64, :],
                                  in_=aT_h[bj][h][:, scol:scol + 512])
                nc.sync.dma_start(out=a2a_in[h][j, 64:65, :],
                                  in_=dens[bj][64:65, h, scol:scol + 512])
            nc.gpsimd.collective_compute(
                "AllToAll", mybir.AluOpType.bypass,
                replica_groups=[list(range(NCORES))],
                ins=[a2a_in[h][:].opt()], outs=[a2a_out[h][:].opt()])
    persA.release()

    # ======================================================================
    # Phases D+E fused scope: projection + residual + LN2 + FFN + output
    # ======================================================================
    persD = tc.alloc_tile_pool(name="persD", bufs=1)
    x2 = persD.tile([128, 4, C], f32, name="x2")
    h2T = persD.tile([128, 8, 512], bf16, name="h2T")
    ff1T = persD.tile([128, 32, 512], bf16, name="ff1T")
    w1r = w1[:].rearrange("(cc p) m -> p cc m", p=128)
    with tc.tile_pool(name="prDE", bufs=1) as prD:
        aT_own = prD.tile([128, 8, 512], bf16, tag="aT_own", name="aT_own")
        for r in range(8):
            nc.gpsimd.dma_start(out=aT_own[0:64, r, :],
                                in_=a2a_out[0][r, 0:64, :])
            nc.gpsimd.dma_start(out=aT_own[64:128, r, :],
                                in_=a2a_out[1][r, 0:64, :])
        # receiver-side softmax normalization: r-th block rows scale by
        # 1/den of heads {2r, 2r+1} (denoms rode each A2A as row 64)
        for r in range(8):
            rb = prD.tile([128, 512], bf16, tag="rb", bufs=3, name=f"rb_{r}")
            nc.gpsimd.dma_start(out=rb[0:64, :],
                                in_=a2a_out[0][r, 64:65, :].to_broadcast([64, 512]))
            nc.gpsimd.dma_start(out=rb[64:128, :],
                                in_=a2a_out[1][r, 64:65, :].to_broadcast([64, 512]))
            rc = prD.tile([128, 512], f32, tag="rc", bufs=3, name=f"rc_{r}")
            nc.vector.tensor_copy(out=rc[:], in_=rb[:])
            rf = prD.tile([128, 512], f32, tag="rf", bufs=3, name=f"rf_{r}")
            nc.vector.reciprocal_approx_fast(out=rf[:], in_=rc[:])
            nc.vector.tensor_mul(aT_own[:, r, :], aT_own[:, r, :], rf[:])
        wp_sb = prD.tile([128, 8, C], bf16, tag="wp_sb", name="wp_sb")
        nc.scalar.dma_start(out=wp_sb[:],
                            in_=wproj[:].rearrange("(dc p) e -> p dc e", p=128))
        xo = prD.tile([128, 4, C], f32, tag="xo", name="xo")
        nc.scalar.dma_start(out=xo[:],
                            in_=x_own[:].rearrange("(tq p) e -> p tq e", p=128))
        with tc.tile_pool(name="psD", bufs=1, space="PSUM") as psD, \
             nc.named_scope("proj_ln2"):
            h2subs = []
            mv2t = prD.tile([128, 2, 4], f32, tag="mv2t", name="mv2t")
            for tq in range(4):
                for eh in range(2):
                    pp = psD.tile([128, 512], f32, tag="pp", bufs=2,
                                  name=f"pp_{tq}_{eh}")
                    for dc in range(8):
                        nc.tensor.matmul(
                            pp[:], aT_own[:, dc, tq * 128:(tq + 1) * 128],
                            wp_sb[:, dc, eh * 512:eh * 512 + 512],
                            start=(dc == 0), stop=False)
                    nc.tensor.matmul(pp[:], ones_b_sb[:],
                                     bproj_sb[0:1, eh * 512:eh * 512 + 512],
                                     start=False, stop=True)
                    nc.vector.tensor_add(x2[:, tq, eh * 512:eh * 512 + 512],
                                         pp[:], xo[:, tq, eh * 512:eh * 512 + 512])
                st2 = prD.tile([128, 2, 6], f32, tag="st2", bufs=2,
                               name=f"st2_{tq}")
                nc.vector.bn_stats(out=st2[:, 0, :], in_=x2[:, tq, 0:512])
                nc.vector.bn_stats(out=st2[:, 1, :], in_=x2[:, tq, 512:1024])
                nc.vector.bn_aggr(out=mv2t[:, :, tq], in_=st2[:])
            rsq2 = _rsqrt_dve(nc, mybir, prD, mv2t[:, 1, :], 4, "ln2")
            for tq in range(4):
                h2 = prD.tile([128, C], bf16, tag="h2", bufs=5, name=f"h2_{tq}")
                nc.vector.tensor_scalar(out=h2[:], in0=x2[:, tq, :],
                                        scalar1=mv2t[:, 0, tq:tq + 1],
                                        scalar2=rsq2[:, tq:tq + 1],
                                        op0=OP.subtract, op1=OP.mult)
                h2subs.append(h2)
            for cc in range(8):
                pt2 = psD.tile([128, 512], f32, tag="pp", bufs=2,
                               name=f"pt2_{cc}")
                for tq in range(4):
                    nc.tensor.matmul(pt2[:, tq * 128:(tq + 1) * 128],
                                     h2subs[tq][:, cc * 128:(cc + 1) * 128],
                                     idb_sb[:], start=True, stop=True)
                nc.scalar.copy(out=h2T[:, cc, :], in_=pt2[:])
        with tc.tile_pool(name="ps1", bufs=1, space="PSUM") as ps1, \
             nc.named_scope("ffn1"):
            for w in range(16):  # m-windows of 256
                w1w = prD.tile([128, 8, 256], bf16, tag="w1w", bufs=3,
                               name=f"w1w_{w}")
                nc.gpsimd.dma_start(out=w1w[:], in_=w1r[:, :, w * 256:(w + 1) * 256])
                for m2 in range(2):
                    m = w * 2 + m2  # m-chunk of 128
                    pf = ps1.tile([128, 512], f32, tag="pf", bufs=3,
                                  name=f"pf_{m}")
                    for cc in range(8):
                        nc.tensor.matmul(
                            pf[:], w1w[:, cc, m2 * 128:(m2 + 1) * 128],
                            h2T[:, cc, :], start=(cc == 0), stop=(cc == 7))
                    nc.scalar.activation(out=ff1T[:, m, :], in_=pf[:],
                                         func=AF.Relu, bias=b1_sb[:, m:m + 1])
        # FFN2 with ff1T stationary / w2 rows moving (1024-wide bf16):
        # output lands directly as ff[t, e]; no transposes needed.
        with tc.tile_pool(name="ps2", bufs=1, space="PSUM") as ps2p, \
             nc.named_scope("ffn2"):
            pso = [ps2p.tile([128, C], f32, tag="pso", bufs=4, name=f"pso_{tq}")
                   for tq in range(4)]
            for mc in range(32):
                w2t = prD.tile([128, C], bf16, tag="w2t", bufs=6,
                               name=f"w2t_{mc}")
                nc.gpsimd.dma_start(out=w2t[:], in_=w2[mc * 128:(mc + 1) * 128, :])
                for tq in range(4):
                    for eh in range(2):
                        nc.tensor.matmul(pso[tq][:, eh * 512:(eh + 1) * 512],
                                         ff1T[:, mc, tq * 128:(tq + 1) * 128],
                                         w2t[:, eh * 512:(eh + 1) * 512],
                                         start=(mc == 0), stop=False)
            for tq in range(4):
                for eh in range(2):
                    nc.tensor.matmul(pso[tq][:, eh * 512:(eh + 1) * 512],
                                     ones_b_sb[:], b2r_sb[0:1, eh * 512:(eh + 1) * 512],
                                     start=False, stop=True)
                ot = prD.tile([128, C], f32, tag="ot", bufs=2, name=f"ot_{tq}")
                nc.vector.tensor_add(ot[:], pso[tq][:], x2[:, tq, :])
                nc.sync.dma_start(out=out[tq * 128:(tq + 1) * 128, :], in_=ot[:])
    persD.release()
    consts.release()
    dram.release()


# --------------------------------------------------------------------------
# host driver
# --------------------------------------------------------------------------
def _make_in_maps(inputs):
    x = np.ascontiguousarray(np.asarray(inputs["x"], np.float32))
    wq = np.asarray(inputs["wq"], np.float32)
    wk = np.asarray(inputs["wk"], np.float32)
    wv = np.asarray(inputs["wv"], np.float32)
    w_proj = np.ascontiguousarray(np.asarray(inputs["w_proj"], np.float32))
    b_proj = np.asarray(inputs["b_proj"], np.float32)
    w1 = np.ascontiguousarray(np.asarray(inputs["w1"], np.float32))
    b1 = np.asarray(inputs["b1"], np.float32)
    w2 = np.ascontiguousarray(np.asarray(inputs["w2"], np.float32))
    b2 = np.asarray(inputs["b2"], np.float32)
    g1 = np.asarray(inputs["g1"], np.float32)
    be1 = np.asarray(inputs["be1"], np.float32)
    g2 = np.asarray(inputs["g2"], np.float32)
    be2 = np.asarray(inputs["be2"], np.float32)

    xf = x.reshape(BT, C)
    b1 = b1 + be2 @ w1          # fold LN2 beta (uses original w1)
    w1 = np.ascontiguousarray(g2[:, None] * w1)   # fold LN2 gamma

    i_mask = np.zeros((4, 128, 512), np.float32)
    s_idx = np.arange(128)[:, None]
    t_idx = np.arange(512)[None, :]
    for i in range(4):
        i_mask[i] = (s_idx + 128 * i <= t_idx).astype(np.float32)
    onespat = np.zeros((1, 130), np.float32)
    onespat[0, 64] = 1.0
    onespat[0, 129] = 1.0

    common = dict(
        x_full=xf.astype(ml_dtypes.bfloat16),
        masks=i_mask.astype(ml_dtypes.bfloat16),
        onespat=onespat.astype(ml_dtypes.bfloat16),
        ones_b=np.ones((1, 128), ml_dtypes.bfloat16),
        wproj=w_proj.astype(ml_dtypes.bfloat16),
        bproj=np.ascontiguousarray(b_proj[None, :]).astype(ml_dtypes.bfloat16),
        w1=w1.astype(ml_dtypes.bfloat16), w2=w2.astype(ml_dtypes.bfloat16),
        b1t=np.ascontiguousarray(b1.reshape(FF // 128, 128).T),
        b2row=np.ascontiguousarray(b2[None, :]).astype(ml_dtypes.bfloat16),
        identb=np.eye(128).astype(ml_dtypes.bfloat16),
    )
    in_maps = []
    for c in range(NCORES):
        b, hg = c // 4, c % 4
        wva = np.zeros((C, 130), np.float32)
        wva[:, 0:64] = wv[2 * c]
        wva[:, 65:129] = wv[2 * c + 1]
        wq2 = np.concatenate([wq[2 * c], wq[2 * c + 1]], axis=1)
        wk2 = np.concatenate([wk[2 * c], wk[2 * c + 1]], axis=1)
        m = dict(common)
        m["x_own"] = np.ascontiguousarray(
            xf[b * T + hg * TSL: b * T + (hg + 1) * TSL])
        m["wq2"] = np.ascontiguousarray(g1[:, None] * wq2).astype(
            ml_dtypes.bfloat16)
        m["wk2"] = np.ascontiguousarray(g1[:, None] * wk2).astype(
            ml_dtypes.bfloat16)
        m["wv_aug"] = (g1[:, None] * wva).astype(ml_dtypes.bfloat16)
        m["onespat"] = np.ascontiguousarray(
            (be1 @ wva + onespat[0])[None, :]).astype(ml_dtypes.bfloat16)
        m["cbq"] = np.ascontiguousarray(
            np.stack([be1 @ wq2, be1 @ wk2], axis=1))
        in_maps.append(m)
    return in_maps


LAST_RESULTS = None


def kernel(trace=False, **inputs):
    global LAST_RESULTS
    from concourse import bass_utils

    if "nc" not in _CACHE:
        _CACHE["nc"] = _build_program()
    nc = _CACHE["nc"]
    in_maps = _make_in_maps(inputs)
    res = bass_utils.run_bass_kernel_spmd(
        nc, in_maps, core_ids=list(range(NCORES)), trace=trace)
    LAST_RESULTS = res
    out = np.zeros((B, T, C), np.float32)
    for c in range(NCORES):
        b, hg = c // 4, c % 4
        out[b, hg * TSL:(hg + 1) * TSL, :] = res.results[c]["out"]
    return out
